# revision 51
# baseline (speedup 1.0000x reference)
"""Trainium2 Bass kernel for nn_Encoder_506806141403.

12-layer transformer encoder (D=768, H=12, FF=3072) with the quirk that
attention scores use Q vs V (no K projection) and scale by D**-0.5.

Sharding: 8 cores = 4 batch elements x 2 sequence halves. Each core owns
512 query rows of one batch element. Per layer, each core computes its half's
V projection (token-major, bf16) and the pair exchanges halves via a 2-rank
AllGather, so every core holds all 1024 keys/values of its batch element.
Everything else (LN, Q, scores, softmax, ctx, Wo, FFN) is computed per-core
on its own 512 rows.

v2 performance changes vs. baseline:
- Attention-path GEMMs (V, Q, Wo, probs@V) run in fp8e4 DoubleRow perf mode
  (2x PE throughput; weights host-scaled x64 to stay in fp8 normal range,
  descale folded into the softmax scale / fused evacuation multiplies).
  FFN + scores stay bf16: fp8 there would exceed the 2e-2 error budget.
- Softmax Z: reciprocal_approx_fast on [1,512] rows + a K=2 outer-product
  matmul broadcast (replaces exact DVE reciprocal 3.3us/head + gpsimd
  partition_broadcast 1us/head).
- LayerNorm rstd (exp(-0.5*ln(var+eps))) computed once per 4-block phase on
  [128,4] stats instead of per block, eliminating ACT table thrash.
- LN transposes in bf16 (1.0 cyc/row on PE vs 1.5 for f32r).
- Optional row-tiled scores (64x128 PE tiling, two heads concurrently).
"""
import os
import sys

sys.path.insert(0, "/opt/trn_rl_repo")

import numpy as np
import ml_dtypes

import concourse.bass as bass
from concourse.bass import ds
from concourse import bacc
import concourse.tile as tile
from concourse import mybir
from concourse.bass_utils import run_bass_kernel_spmd

P = 128
D = 768
H = 12
DH = 64
FF = 3072
NB_D = 6          # D / P
NB_T = 4          # own tokens 512 / P
NB_K = 8          # full tokens 1024 / P
NB_FF = 24        # FF / P
T_OWN = 512
SCALE = float(D) ** -0.5
LN_EPS = 1e-5
N_LAYERS = int(os.environ.get("KERNEL_N_LAYERS", "12"))
FP8_ATTN = bool(int(os.environ.get("KERNEL_FP8_ATTN", "1")))
SCORES_RT = bool(int(os.environ.get("KERNEL_SCORES_RT", "1")))
WS = 64.0 if FP8_ATTN else 1.0   # host-side weight scale for fp8 range

F32 = mybir.dt.float32
F32R = mybir.dt.float32r
BF16 = mybir.dt.bfloat16
FP8 = mybir.dt.float8e4
AF = mybir.ActivationFunctionType
OP = mybir.AluOpType
DR = mybir.MatmulPerfMode.DoubleRow

A8 = FP8 if FP8_ATTN else BF16   # attention-path activation dtype

REPLICA_GROUPS = [[0, 1], [2, 3], [4, 5], [6, 7]]

_cached = {}
_last_results = None


def _register_ntff_hook():
    """Register the axon NTFF profile hook (for trace=True exec timing)."""
    import types
    try:
        import antenv.axon_hooks  # noqa: F401
        return
    except ImportError:
        pass
    try:
        from trn_agent_boot.trn_boot import _ntff_profile_via_ctypes
        import antenv
        hook = _ntff_profile_via_ctypes("/opt/axon/libaxon_pjrt.so")
        mod = types.ModuleType("antenv.axon_hooks")
        mod.get_axon_ntff_profile_hook = lambda: hook
        mod.set_axon_ntff_profile_hook = lambda h: None
        sys.modules["antenv.axon_hooks"] = mod
        antenv.axon_hooks = mod
    except Exception:
        pass


def _regions():
    return ((0, 512), (512, 768))


def build(n_layers=N_LAYERS, zero_bias=False):
    nc = bacc.Bacc(None, target_bir_lowering=False, num_devices=8)
    L = n_layers
    W8 = FP8 if FP8_ATTN else BF16

    x_d = nc.dram_tensor("x", [P, NB_T, D], F32R, kind="ExternalInput")
    wq_d = nc.dram_tensor("wq", [L, P, NB_D * D], W8, kind="ExternalInput")
    wv_d = nc.dram_tensor("wv", [L, P, NB_D * D], W8, kind="ExternalInput")
    wo_d = nc.dram_tensor("wo", [L, P, NB_D * D], W8, kind="ExternalInput")
    w1_d = nc.dram_tensor("w1", [L, 4, P, NB_D * D], BF16, kind="ExternalInput")
    w2_d = nc.dram_tensor("w2", [L, 4, P, NB_D * D], BF16, kind="ExternalInput")
    bq_d = nc.dram_tensor("bq", [P, L, NB_D], F32, kind="ExternalInput")
    bvf_d = nc.dram_tensor("bvf", [P, L, NB_D], F32, kind="ExternalInput")
    b1_d = nc.dram_tensor("b1", [P, L, NB_FF], F32, kind="ExternalInput")
    bvr_d = nc.dram_tensor("bv_row", [1, L, D], F32, kind="ExternalInput")
    bor_d = nc.dram_tensor("bo_row", [1, L, D], BF16, kind="ExternalInput")
    b2r_d = nc.dram_tensor("b2_row", [1, L, D], BF16, kind="ExternalInput")
    idbf_d = nc.dram_tensor("identbf", [P, P], BF16, kind="ExternalInput")
    ones_d = nc.dram_tensor("ones1", [1, P], BF16, kind="ExternalInput")
    sel2_d = nc.dram_tensor("sel2", [2, P], BF16, kind="ExternalInput")
    out_d = nc.dram_tensor("out", [P, NB_T, D], F32R, kind="ExternalOutput")

    with tile.TileContext(nc) as tc:
        with (
            tc.tile_pool(name="state", bufs=1) as st,
            tc.tile_pool(name="stream", bufs=2) as sp,
            tc.tile_pool(name="acts", bufs=1) as ap,
            tc.tile_pool(name="wpool", bufs=2) as wp,
            tc.tile_pool(name="psA", bufs=2, space="PSUM") as psA,
            tc.tile_pool(name="psB", bufs=4, space="PSUM") as psB,
            tc.tile_pool(name="dram", bufs=2, space="DRAM") as dp,
        ):
            # ---- initial stream first: x gates layer 0 ----
            x_t = sp.tile([P, NB_T, D], F32R, tag="stream", name="x0")
            nc.sync.dma_start(x_t[:], x_d[:])

            # ---- constants ----
            idbf = st.tile([P, P], BF16)
            ones1 = st.tile([1, P], BF16)
            sel_a = st.tile([1, P], BF16)
            sel_b = st.tile([1, P], BF16)
            bq_all = st.tile([P, L, NB_D], F32)
            bvf_all = st.tile([P, L, NB_D], F32)
            b1_all = st.tile([P, L, NB_FF], F32)
            eps_t = st.tile([P, 1], F32)
            q_par = [st.tile([P, NB_D, T_OWN], A8, name=f"qpar{p}") for p in range(2)]
            if not SCORES_RT:
                nc.vector.memset(q_par[0][:], 0.0)
                nc.vector.memset(q_par[1][:], 0.0)
            nc.sync.dma_start(idbf[:], idbf_d[:])
            nc.sync.dma_start(ones1[:], ones_d[:])
            nc.sync.dma_start(sel_a[:], sel2_d[0:1, :])
            nc.sync.dma_start(sel_b[:], sel2_d[1:2, :])
            nc.sync.dma_start(bq_all[:], bq_d[:])
            nc.sync.dma_start(bvf_all[:], bvf_d[:])
            nc.sync.dma_start(b1_all[:], b1_d[:])
            nc.vector.memset(eps_t[:], LN_EPS)

            def ln_stats(in_ap, mv_ap):
                """bn stats for one [128, 768] block -> mv_ap [128, 2]."""
                t = ap.tile([P, 2, 6], F32, tag="lnscratch", bufs=3, name="lnt")
                xg = in_ap.rearrange("p (g d) -> p g d", g=2)
                for g in range(2):
                    nc.vector.bn_stats(t[:, g, :], xg[:, g, :])
                nc.vector.bn_aggr(mv_ap, t[:])

            def ln_rstd(mv4, nblk, name):
                """rstd [128, nblk] = (var + eps) ** -0.5, batched.

                Sqrt on ACT (one table) + fast reciprocal on DVE keeps the
                critical path to a single activation-table dependency."""
                std = ap.tile([P, nblk], F32, tag="lnt2", bufs=2, name=f"lns{name}")
                rstd = ap.tile([P, nblk], F32, tag="rstd", bufs=2, name=f"rstd{name}")
                nc.scalar.activation(
                    std[:], mv4[:, :, 1:2].rearrange("p a b -> p (a b)"),
                    AF.Sqrt, bias=eps_t[:], scale=1.0,
                )
                nc.vector.reciprocal_approx_fast(rstd[:], std[:])
                return rstd

            def sqrt_prefetch(name):
                """Dummy Sqrt so the table load lands while ACT is idle."""
                t = ap.tile([P, 1], F32, tag="sqpf", bufs=2, name=name)
                nc.scalar.activation(t[:], eps_t[:], AF.Sqrt, scale=1.0)

            def ln_apply(out_ap, in_ap, mv_ap, rstd_ap):
                nc.vector.tensor_scalar(
                    out=out_ap, in0=in_ap, scalar1=mv_ap[:, 0:1], scalar2=rstd_ap,
                    op0=OP.subtract, op1=OP.mult,
                )

            def transpose_tm_to_fm(h_tm, dtype, tag, name):
                """[128, 4, 768] token-major bf16 -> [128, 6, 512] feature-major.

                Per-token-block (tb) granularity so each block transposes as
                soon as its LN apply lands, pipelining into the previous
                phase instead of serializing at the layer boundary."""
                h_fm = ap.tile([P, NB_D, T_OWN], dtype, tag=tag, bufs=1, name=name)
                for tb in range(NB_T):
                    trp = psB.tile([P, NB_D * P], BF16, tag="psB",
                                   name=f"{name}_tr{tb}")
                    for db in range(NB_D):
                        nc.tensor.transpose(
                            trp[:, db * P:(db + 1) * P],
                            h_tm[:, tb, db * P:(db + 1) * P], idbf[:],
                        )
                    nc.vector.tensor_copy(
                        h_fm[:, :, tb * P:(tb + 1) * P],
                        trp[:].rearrange("p (d c) -> p d c", c=P),
                    )
                return h_fm

            for l in range(L):
                # ---- weights for this layer ----
                wq = wp.tile([P, NB_D, D], W8, tag="w8", bufs=4, name=f"wq{l}")
                nc.sync.dma_start(wq[:], wq_d[l].rearrange("p (k n) -> p k n", n=D))
                wv = wp.tile([P, NB_D, D], W8, tag="w8", bufs=4, name=f"wv{l}")
                nc.sync.dma_start(wv[:], wv_d[l].rearrange("p (k n) -> p k n", n=D))
                if not zero_bias:
                    bv_row = ap.tile([1, D], F32, tag="bvrow", bufs=2, name=f"bvr{l}")
                    nc.sync.dma_start(bv_row[:], bvr_d[:, l, :])
                    bo_row = ap.tile([1, D], BF16, tag="borow", bufs=2, name=f"bor{l}")
                    nc.sync.dma_start(bo_row[:], bor_d[:, l, :])
                    b2_row = ap.tile([1, D], BF16, tag="b2row", bufs=2, name=f"b2r{l}")
                    nc.sync.dma_start(b2_row[:], b2r_d[:, l, :])

                # ---- LN1 (batched rstd) + transpose ----
                with nc.named_scope(f"L{l:02d}_a_ln1"):
                    h_tm = ap.tile([P, NB_T, D], BF16, tag="h_tm", bufs=1,
                                   name=f"h1tm{l}")
                    mv4 = ap.tile([P, NB_T, 2], F32, tag="mv4", bufs=2,
                                  name=f"mv1_{l}")
                    for tb in range(NB_T):
                        ln_stats(x_t[:, tb, :], mv4[:, tb, :])
                    rstd = ln_rstd(mv4, NB_T, f"a{l}")
                    for tb in range(NB_T):
                        ln_apply(h_tm[:, tb, :], x_t[:, tb, :], mv4[:, tb, :],
                                 rstd[:, tb:tb + 1])
                    h1_fm = transpose_tm_to_fm(h_tm, A8, "h1_fm", f"h1fm{l}")

                def mm_pairs(out_ap, lhs_fn, rhs_fn, nk, extra_mm=None):
                    """Contraction over nk 128-blocks; fp8 uses DoubleRow pairs."""
                    if FP8_ATTN:
                        npair = nk // 2
                        for k in range(npair):
                            nc.tensor.matmul(
                                out_ap, lhs_fn(2 * k, 2), rhs_fn(2 * k, 2),
                                start=(k == 0),
                                stop=(k == npair - 1 and extra_mm is None),
                                perf_mode=DR,
                            )
                    else:
                        for k in range(nk):
                            nc.tensor.matmul(
                                out_ap, lhs_fn(k, 1), rhs_fn(k, 1),
                                start=(k == 0),
                                stop=(k == nk - 1 and extra_mm is None),
                            )
                    if extra_mm is not None:
                        extra_mm()

                # ---- V token-major (+bias) -> send buffer ----
                nc.enter_named_scope(f"L{l:02d}_b_v", False)
                if not zero_bias:
                    bv_bc = ap.tile([P, D], F32, tag="bv_bc", bufs=1, name=f"bvbc{l}")
                    nc.gpsimd.partition_broadcast(bv_bc[:], bv_row[:])
                v_send = ap.tile([P, NB_T, D], A8, tag="vsend", bufs=1,
                                 name=f"vsend{l}")
                for tb in range(NB_T):
                    vp = psA.tile([P, D], F32, tag="psA", name=f"vps{l}_{tb}")
                    for n0, n1 in _regions():
                        mm_pairs(
                            vp[:, n0:n1],
                            lambda k, w: h1_fm[:, k:k + w, tb * P:(tb + 1) * P],
                            lambda k, w: wv[:, k:k + w, n0:n1],
                            NB_D,
                        )
                    if zero_bias:
                        nc.vector.tensor_copy(v_send[:, tb, :], vp[:])
                    else:
                        nc.vector.tensor_tensor(
                            v_send[:, tb, :], vp[:], bv_bc[:], OP.add
                        )

                # own half of v_fm directly as feature-major matmuls (no
                # transposes needed; also what the peer receives pre-made)
                v_fm = ap.tile([P, NB_D, NB_K * P], A8, tag="v_fm", bufs=1,
                               name=f"vfm{l}")
                vsend2_dr = dp.tile([NB_D, P, T_OWN], A8, tag="vsend2_d",
                                    name=f"vs2dr{l}")
                for db in range(NB_D):
                    vq = psB.tile([P, T_OWN], F32, tag="psB", name=f"vfps{l}_{db}")
                    mm_pairs(
                        vq[:],
                        lambda k, w: wv[:, k:k + w, db * P:(db + 1) * P],
                        lambda k, w: h1_fm[:, k:k + w, :],
                        NB_D,
                    )
                    nc.vector.tensor_scalar(
                        out=v_fm[:, db, 0:T_OWN], in0=vq[:],
                        scalar1=bvf_all[:, l, db:db + 1], scalar2=None,
                        op0=OP.add,
                    )
                    nc.sync.dma_start(vsend2_dr[db], v_fm[:, db, 0:T_OWN])

                nc.leave_named_scope(f"L{l:02d}_b_v", None, False)
                nc.enter_named_scope(f"L{l:02d}_c_ag", False)
                vsend_dr = dp.tile([NB_T, P, D], A8, tag="vsend_d", name=f"vsdr{l}")
                for tb in range(NB_T):
                    nc.sync.dma_start(vsend_dr[tb], v_send[:, tb, :])
                vrecv_dr = dp.tile([NB_K, P, D], A8, tag="vrecv_d", name=f"vrdr{l}")
                vrecv2_dr = dp.tile([2 * NB_D, P, T_OWN], A8, tag="vrecv2_d",
                                    name=f"vr2dr{l}")
                nc.gpsimd.collective_compute(
                    "AllGather", OP.bypass, replica_groups=REPLICA_GROUPS,
                    ins=[vsend_dr[:]], outs=[vrecv_dr[:]],
                )
                nc.gpsimd.collective_compute(
                    "AllGather", OP.bypass, replica_groups=REPLICA_GROUPS,
                    ins=[vsend2_dr[:]], outs=[vrecv2_dr[:]],
                )
                nc.leave_named_scope(f"L{l:02d}_c_ag", None, False)
                nc.enter_named_scope(f"L{l:02d}_d_q", False)
                # ---- Q (overlaps the AllGather) ----
                for m in range(NB_D):
                    qp = psB.tile([P, T_OWN], F32, tag="psB", name=f"qps{l}_{m}")
                    mm_pairs(
                        qp[:],
                        lambda k, w: wq[:, k:k + w, m * P:(m + 1) * P],
                        lambda k, w: h1_fm[:, k:k + w, :],
                        NB_D,
                    )
                    for hh in range(2):
                        r0 = 64 * hh
                        nc.vector.tensor_scalar(
                            out=q_par[hh][r0:r0 + 64, m, :], in0=qp[r0:r0 + 64, :],
                            scalar1=bq_all[r0:r0 + 64, l, m:m + 1], scalar2=None,
                            op0=OP.add,
                        )

                # prefetch Wo while attention runs
                wo = wp.tile([P, NB_D, D], W8, tag="w8", bufs=4, name=f"wo{l}")
                nc.sync.dma_start(wo[:], wo_d[l].rearrange("p (k n) -> p k n", n=D))

                nc.leave_named_scope(f"L{l:02d}_d_q", None, False)
                nc.enter_named_scope(f"L{l:02d}_e_vrecv", False)
                # ---- receive V: augmented token-major (A8) + feature-major bf16 ----
                # own-relative key order: kb 0-3 = own half (local data),
                # kb 4-7 = peer half (dynamic shard pick from vrecv)
                # row stride padded to 784 (16-aligned) for DoubleRow LDWEIGHTS
                v_aug_full = ap.tile([P, NB_K, 784], A8, tag="v_aug", bufs=1,
                                     name=f"vaug{l}")
                v_aug = v_aug_full[:, :, 0:H * 65]
                va4 = v_aug.rearrange("p k (h c) -> p k h c", c=65)
                for kb in range(NB_K):
                    nc.vector.memset(va4[:, kb, :, 64:65], 1.0)
                for tb in range(NB_T):
                    nc.vector.tensor_copy(
                        va4[:, tb, :, 0:64],
                        v_send[:, tb, :].rearrange("p (h c) -> p h c", c=64),
                    )
                peer_off = (1 - (nc.gpsimd.partition_id() % 2)) * NB_T
                vr4 = vrecv_dr.rearrange("k p (h c) -> k p h c", c=64)
                for tb in range(NB_T):
                    nc.gpsimd.dma_start(
                        va4[:, 4 + tb, :, 0:64], vr4[ds(peer_off + tb, 1)],
                    )
                # peer half of v_fm arrives pre-transposed from the AllGather
                peer_off2 = (1 - (nc.gpsimd.partition_id() % 2)) * NB_D
                for db in range(NB_D):
                    nc.gpsimd.dma_start(
                        v_fm[:, db, T_OWN:], vrecv2_dr[ds(peer_off2 + db, 1)],
                    )
                nc.leave_named_scope(f"L{l:02d}_e_vrecv", None, False)
                nc.enter_named_scope(f"L{l:02d}_f_attn", False)
                # ---- attention: flat software pipeline over (head-pair, kb-pair)
                # steps; ctx lags scores by one step; db 0-1 own-half steps overlap
                # the AllGather ----
                ctx_n = ap.tile([P, NB_D, T_OWN], A8, tag="ctx_n", bufs=1,
                                name=f"ctxn{l}")
                OWN_PAIRS = [(0, 1), (2, 3)]
                PEER_PAIRS = [(4, 5), (6, 7)]

                ctxps = {}
                ex_own = {}
                pending = None  # (db, pair, ex, is_first_peer, is_last)

                def score_mms(db, pair, spv):
                    for o in range(2):
                        kb = pair[o]
                        for hh in range(2):
                            if SCORES_RT:
                                r0 = 64 * hh
                                nc.tensor.matmul(
                                    spv[hh][:, o * 512:(o + 1) * 512],
                                    v_fm[r0:r0 + 64, db, kb * P:(kb + 1) * P],
                                    q_par[hh][r0:r0 + 64, db, :],
                                    start=True, stop=True,
                                )
                            else:
                                nc.tensor.matmul(
                                    spv[hh][:, o * 512:(o + 1) * 512],
                                    v_fm[:, db, kb * P:(kb + 1) * P],
                                    q_par[hh][:, db, :],
                                    start=True, stop=True,
                                )

                def exp_tiles(l, db, pair, spv, tag, bufs):
                    ex = [
                        ap.tile([P, 2, 512], A8, tag=tag, bufs=bufs,
                                name=f"ex{l}_{db}_{pair[0]}_{hh}")
                        for hh in range(2)
                    ]
                    for hh in range(2):
                        nc.scalar.activation(
                            ex[hh].rearrange("p a b -> p (a b)"), spv[hh][:],
                            AF.Exp, scale=SCALE / (WS * WS),
                        )
                    return ex

                def ctx_mm(db, pair, ex, start, stop):
                    hpair = (2 * db, 2 * db + 1)
                    for hh in range(2):
                        h = hpair[hh]
                        if FP8_ATTN:
                            nc.tensor.matmul(
                                ctxps[db][hh][:],
                                v_aug[:, pair[0]:pair[0] + 2, 65 * h:65 * h + 65],
                                ex[hh][:],
                                start=start, stop=stop,
                                perf_mode=DR,
                            )
                        else:
                            for o in range(2):
                                nc.tensor.matmul(
                                    ctxps[db][hh][:],
                                    v_aug[:, pair[o], 65 * h:65 * h + 65],
                                    ex[hh][:, o, :],
                                    start=(start and o == 0), stop=(stop and o == 1),
                                )

                def flush_ctx():
                    nonlocal pending
                    if pending is None:
                        return
                    db, pair, ex, first, last = pending
                    if first:
                        ctxps[db] = [
                            psB.tile([65, T_OWN], F32, tag="psB",
                                     name=f"ctxp{l}_{2 * db + hh}")
                            for hh in range(2)
                        ]
                        for j, opair in enumerate(OWN_PAIRS):
                            ctx_mm(db, opair, ex_own.pop((db, opair)),
                                   start=(j == 0), stop=False)
                    ctx_mm(db, pair, ex, start=False, stop=last)
                    if last:
                        # Z rows -> bf16 sbuf, broadcast both heads into one
                        # [128,512] psum via two accumulating K=1 outer products
                        # on the PE, one 128-lane reciprocal, then one DVE
                        # multiply per head.
                        zrow = [
                            ap.tile([1, T_OWN], BF16, tag="zrow", bufs=4,
                                    name=f"zr{l}_{db}_{hh}")
                            for hh in range(2)
                        ]
                        for hh in range(2):
                            nc.vector.tensor_copy(
                                zrow[hh][:], ctxps[db][hh][64:65, :]
                            )
                        zbcp = psA.tile([P, T_OWN], F32, tag="psA",
                                        name=f"zbp{l}_{db}")
                        nc.tensor.matmul(
                            zbcp[:], sel_a[:], zrow[0][:],
                            start=True, stop=False,
                        )
                        nc.tensor.matmul(
                            zbcp[:], sel_b[:], zrow[1][:],
                            start=False, stop=True,
                        )
                        zbci = ap.tile([P, T_OWN], F32, tag="zbc", bufs=2,
                                       name=f"zb{l}_{db}")
                        nc.vector.reciprocal_approx_fast(zbci[:], zbcp[:])
                        for hh in range(2):
                            r0 = 64 * hh
                            nc.vector.tensor_tensor(
                                ctx_n[r0:r0 + 64, db, :], ctxps[db][hh][0:64, :],
                                zbci[r0:r0 + 64, :], OP.mult,
                            )
                        del ctxps[db]
                    pending = None

                # phase 1: own-half scores+exp for every head pair, buffered.
                # Runs during the AllGather window (needs only v_fm own half
                # and q_par), keeping the PE warm while the collective runs.
                for db in range(NB_D):
                    for pair in OWN_PAIRS:
                        spv = [
                            psA.tile([P, 1024], F32, tag="psA",
                                     name=f"sc{l}_{db}_{pair[0]}_{hh}")
                            for hh in range(2)
                        ]
                        score_mms(db, pair, spv)
                        ex_own[(db, pair)] = exp_tiles(l, db, pair, spv,
                                                       "expOwn", 24)

                # phase 2: peer-half scores+exp pipelined with ctx accumulation
                # (own-half ctx MMs issue from the buffered exps).
                for db in range(NB_D):
                    for pi, pair in enumerate(PEER_PAIRS):
                        spv = [
                            psA.tile([P, 1024], F32, tag="psA",
                                     name=f"sc{l}_{db}_{pair[0]}_{hh}")
                            for hh in range(2)
                        ]
                        score_mms(db, pair, spv)
                        flush_ctx()
                        ex = exp_tiles(l, db, pair, spv, "expT", 5)
                        pending = (db, pair, ex, pi == 0,
                                   pi == len(PEER_PAIRS) - 1)
                flush_ctx()
                nc.leave_named_scope(f"L{l:02d}_f_attn", None, False)
                nc.enter_named_scope(f"L{l:02d}_g_wo", False)
                # ---- Wo + residual; LN2 stats per block as it completes ----
                sqrt_prefetch(f"sqp2_{l}")
                skip = sp.tile([P, NB_T, D], F32R, tag="stream", name=f"skip{l}")
                mv4b = ap.tile([P, NB_T, 2], F32, tag="mv4", bufs=2, name=f"mv2_{l}")
                for lb in range(NB_T):
                    wps = psA.tile([P, D], F32, tag="psA", name=f"wops{l}_{lb}")
                    for n0, n1 in _regions():
                        mm_pairs(
                            wps[:, n0:n1],
                            lambda k, w: ctx_n[:, k:k + w, lb * P:(lb + 1) * P],
                            lambda k, w: wo[:, k:k + w, n0:n1],
                            NB_D,
                            extra_mm=None if zero_bias else (
                                lambda n0=n0, n1=n1: nc.tensor.matmul(
                                    wps[:, n0:n1], ones1[:], bo_row[:, n0:n1],
                                    start=False, stop=True,
                                )
                            ),
                        )
                    if FP8_ATTN:
                        nc.vector.scalar_tensor_tensor(
                            out=skip[:, lb, :], in0=wps[:], scalar=1.0 / (WS * WS),
                            in1=x_t[:, lb, :], op0=OP.mult, op1=OP.add,
                        )
                    else:
                        nc.vector.tensor_tensor(
                            skip[:, lb, :], x_t[:, lb, :], wps[:], OP.add
                        )
                    ln_stats(skip[:, lb, :], mv4b[:, lb, :])
                nc.leave_named_scope(f"L{l:02d}_g_wo", None, False)
                nc.enter_named_scope(f"L{l:02d}_h_ln2", False)
                # ---- LN2 apply + transpose ----
                h_tm2 = ap.tile([P, NB_T, D], BF16, tag="h_tm", bufs=1,
                                name=f"h2tm{l}")
                rstd2 = ln_rstd(mv4b, NB_T, f"b{l}")
                for lb in range(NB_T):
                    ln_apply(h_tm2[:, lb, :], skip[:, lb, :], mv4b[:, lb, :],
                             rstd2[:, lb:lb + 1])
                h2_fm = transpose_tm_to_fm(h_tm2, BF16, "h2_fm", f"h2fm{l}")
                nc.leave_named_scope(f"L{l:02d}_h_ln2", None, False)
                nc.enter_named_scope(f"L{l:02d}_i_ff", False)
                # ---- FFN: FF1 (all 24 hidden blocks) then FF2 in PSUM (bf16) ----
                g_all = ap.tile([P, NB_FF, T_OWN], BF16, tag="g", bufs=1, name=f"g{l}")
                w2cs = []
                for ck in range(4):
                    w1c = wp.tile([P, NB_D, D], BF16, tag="wbf", bufs=5,
                                  name=f"w1c{l}_{ck}")
                    nc.sync.dma_start(w1c[:], w1_d[l, ck].rearrange("p (k n) -> p k n", n=D))
                    w2c = wp.tile([P, NB_D, D], BF16, tag="wbf", bufs=5,
                                  name=f"w2c{l}_{ck}")
                    nc.sync.dma_start(w2c[:], w2_d[l, ck].rearrange("p (k n) -> p k n", n=D))
                    w2cs.append(w2c)
                    for mm in range(NB_D):
                        fp = psB.tile([P, T_OWN], F32, tag="psB", name=f"f1ps{l}_{ck}_{mm}")
                        for kb in range(NB_D):
                            nc.tensor.matmul(
                                fp[:], w1c[:, kb, mm * P:(mm + 1) * P], h2_fm[:, kb, :],
                                start=(kb == 0), stop=(kb == NB_D - 1),
                            )
                        nc.scalar.activation(
                            g_all[:, 6 * ck + mm, :], fp[:], AF.Gelu,
                            bias=b1_all[:, l, 6 * ck + mm:6 * ck + mm + 1], scale=1.0,
                        )
                sqrt_prefetch(f"sqp1_{l}")
                for half in range(2):
                    f2s = []
                    for lb in (2 * half, 2 * half + 1):
                        f2 = psA.tile([P, D], F32, tag="psA", name=f"f2ps{l}_{lb}")
                        f2s.append(f2)
                        for n0, n1 in _regions():
                            for ck in range(4):
                                for mm in range(NB_D):
                                    last = zero_bias and ck == 3 and mm == NB_D - 1
                                    nc.tensor.matmul(
                                        f2[:, n0:n1],
                                        g_all[:, 6 * ck + mm, lb * P:(lb + 1) * P],
                                        w2cs[ck][:, mm, n0:n1],
                                        start=(ck == 0 and mm == 0), stop=last,
                                    )
                            if not zero_bias:
                                nc.tensor.matmul(
                                    f2[:, n0:n1], ones1[:], b2_row[:, n0:n1],
                                    start=False, stop=True,
                                )
                    for i, lb in enumerate((2 * half, 2 * half + 1)):
                        nc.vector.tensor_tensor(
                            skip[:, lb, :], skip[:, lb, :], f2s[i][:], OP.add,
                        )
                nc.leave_named_scope(f"L{l:02d}_i_ff", None, False)
                x_t = skip

            nc.sync.dma_start(out_d[:], x_t[:])
    nc.compile()
    return nc


def _preprocess(inputs, n_layers):
    """Fold LN affine into projections; lay out weights for tile DMA."""
    f32 = np.float32
    L = n_layers
    Wq = np.asarray(inputs["Wq"], f32)[:L]
    Wv = np.asarray(inputs["Wv"], f32)[:L]
    Wo = np.asarray(inputs["Wo"], f32)[:L]
    W1 = np.asarray(inputs["W1"], f32)[:L]
    W2 = np.asarray(inputs["W2"], f32)[:L]
    g1 = np.asarray(inputs["ln1_g"], f32)[:L]
    b1ln = np.asarray(inputs["ln1_b"], f32)[:L]
    g2 = np.asarray(inputs["ln2_g"], f32)[:L]
    b2ln = np.asarray(inputs["ln2_b"], f32)[:L]
    bq = np.asarray(inputs["bq"], f32)[:L]
    bv = np.asarray(inputs["bv"], f32)[:L]
    bo = np.asarray(inputs["bo"], f32)[:L]
    b1 = np.asarray(inputs["b1"], f32)[:L]
    b2 = np.asarray(inputs["b2"], f32)[:L]

    Wq_eff = g1[:, :, None] * Wq
    bq_eff = bq + np.einsum("ld,ldo->lo", b1ln, Wq)
    Wv_eff = g1[:, :, None] * Wv
    bv_eff = bv + np.einsum("ld,ldo->lo", b1ln, Wv)
    W1_eff = g2[:, :, None] * W1
    b1_eff = b1 + np.einsum("ld,ldo->lo", b2ln, W1)

    def fm_weight(W):  # [L, D, D] -> [L, 128, 6*768] with [p, k, n]
        return np.ascontiguousarray(
            W.reshape(L, NB_D, P, D).transpose(0, 2, 1, 3).reshape(L, P, NB_D * D)
        )

    bf = ml_dtypes.bfloat16
    f8 = ml_dtypes.float8_e4m3
    w8 = f8 if FP8_ATTN else bf
    wq_h = fm_weight(Wq_eff * WS).astype(w8)
    wv_h = fm_weight(Wv_eff * WS).astype(w8)
    wo_h = fm_weight(Wo * WS).astype(w8)
    w1_h = np.ascontiguousarray(
        W1_eff.reshape(L, NB_D, P, 4, D).transpose(0, 3, 2, 1, 4).reshape(L, 4, P, NB_D * D)
    ).astype(bf)
    w2_h = np.ascontiguousarray(
        W2.reshape(L, 4, NB_D, P, D).transpose(0, 1, 3, 2, 4).reshape(L, 4, P, NB_D * D)
    ).astype(bf)
    bq_h = np.ascontiguousarray((bq_eff * WS).reshape(L, NB_D, P).transpose(2, 0, 1))
    bvf_h = np.ascontiguousarray((bv_eff * WS).reshape(L, NB_D, P).transpose(2, 0, 1))
    b1_h = np.ascontiguousarray(b1_eff.reshape(L, NB_FF, P).transpose(2, 0, 1))

    sel2 = np.zeros((2, P), f32)
    sel2[0, 0:64] = 1.0
    sel2[1, 64:128] = 1.0
    sel2 = sel2.astype(bf)

    return {
        "wq": wq_h, "wv": wv_h, "wo": wo_h, "w1": w1_h, "w2": w2_h,
        "bq": bq_h, "bvf": bvf_h, "b1": b1_h,
        "bv_row": np.ascontiguousarray(bv_eff[None] * WS),
        "bo_row": np.ascontiguousarray(bo[None] * WS * WS).astype(bf),
        "b2_row": np.ascontiguousarray(b2[None]).astype(bf),
        "identbf": np.eye(P).astype(bf),
        "ones1": np.ones((1, P)).astype(bf),
        "sel2": sel2,
    }


def kernel(**inputs) -> np.ndarray:
    n_layers = N_LAYERS
    zero_bias = not any(
        np.any(np.asarray(inputs[k])) for k in ("bv", "bo", "b2", "ln1_b")
    )
    key = ("nc", n_layers, zero_bias)
    if key not in _cached:
        _cached[key] = build(n_layers, zero_bias)
    nc = _cached[key]

    shared = _preprocess(inputs, n_layers)
    x = np.asarray(inputs["x"], np.float32)  # [4, 1024, 768]
    B, T, _ = x.shape

    in_maps = []
    for c in range(8):
        b, half = c // 2, c % 2
        x_own = x[b, half * T_OWN:(half + 1) * T_OWN]          # [512, 768]
        x_tile = np.ascontiguousarray(
            x_own.reshape(NB_T, P, D).transpose(1, 0, 2)        # [128, 4, 768]
        )
        in_maps.append({**shared, "x": x_tile})

    trace = bool(int(os.environ.get("KERNEL_TRACE", "0")))
    if trace:
        _register_ntff_hook()
    res = run_bass_kernel_spmd(nc, in_maps, core_ids=list(range(8)), trace=trace)
    global _last_results
    _last_results = res

    out = np.empty((B, T, D), dtype=np.float32)
    for c in range(8):
        b, half = c // 2, c % 2
        o = res.results[c]["out"]                               # [128, 4, 768]
        out[b, half * T_OWN:(half + 1) * T_OWN] = (
            o.transpose(1, 0, 2).reshape(T_OWN, D)
        )
    return out


# revision 52
# speedup vs baseline: 1.2573x; 1.2573x over previous
"""Trainium2 Bass kernel for nn_Encoder_506806141403.

12-layer transformer encoder (D=768, H=12, FF=3072) with the quirk that
attention scores use Q vs V (no K projection) and scale by D**-0.5.

Sharding: 8 cores = 4 batch elements x 2 sequence halves. Each core owns
512 query rows of one batch element. Per layer, each core computes its half's
V projection (token-major, bf16) and the pair exchanges halves via a 2-rank
AllGather, so every core holds all 1024 keys/values of its batch element.
Everything else (LN, Q, scores, softmax, ctx, Wo, FFN) is computed per-core
on its own 512 rows.

v2 performance changes vs. baseline:
- Attention-path GEMMs (V, Q, Wo, probs@V) run in fp8e4 DoubleRow perf mode
  (2x PE throughput; weights host-scaled x64 to stay in fp8 normal range,
  descale folded into the softmax scale / fused evacuation multiplies).
  FFN + scores stay bf16: fp8 there would exceed the 2e-2 error budget.
- Softmax Z: reciprocal_approx_fast on [1,512] rows + a K=2 outer-product
  matmul broadcast (replaces exact DVE reciprocal 3.3us/head + gpsimd
  partition_broadcast 1us/head).
- LayerNorm rstd (exp(-0.5*ln(var+eps))) computed once per 4-block phase on
  [128,4] stats instead of per block, eliminating ACT table thrash.
- LN transposes in bf16 (1.0 cyc/row on PE vs 1.5 for f32r).
- Optional row-tiled scores (64x128 PE tiling, two heads concurrently).
"""
import os
import sys

sys.path.insert(0, "/opt/trn_rl_repo")

import numpy as np
import ml_dtypes

import concourse.bass as bass
from concourse.bass import ds
from concourse import bacc
import concourse.tile as tile
from concourse import mybir
from concourse.bass_utils import run_bass_kernel_spmd

P = 128
D = 768
H = 12
DH = 64
FF = 3072
NB_D = 6          # D / P
NB_T = 4          # own tokens 512 / P
NB_K = 8          # full tokens 1024 / P
NB_FF = 24        # FF / P
T_OWN = 512
SCALE = float(D) ** -0.5
LN_EPS = 1e-5
N_LAYERS = int(os.environ.get("KERNEL_N_LAYERS", "12"))
FP8_ATTN = bool(int(os.environ.get("KERNEL_FP8_ATTN", "1")))
SCORES_RT = bool(int(os.environ.get("KERNEL_SCORES_RT", "0")))
WS = 64.0 if FP8_ATTN else 1.0   # host-side weight scale for fp8 range

F32 = mybir.dt.float32
F32R = mybir.dt.float32r
BF16 = mybir.dt.bfloat16
FP8 = mybir.dt.float8e4
AF = mybir.ActivationFunctionType
OP = mybir.AluOpType
DR = mybir.MatmulPerfMode.DoubleRow

A8 = FP8 if FP8_ATTN else BF16   # attention-path activation dtype

REPLICA_GROUPS = [[0, 1], [2, 3], [4, 5], [6, 7]]

_cached = {}
_last_results = None


def _register_ntff_hook():
    """Register the axon NTFF profile hook (for trace=True exec timing)."""
    import types
    try:
        import antenv.axon_hooks  # noqa: F401
        return
    except ImportError:
        pass
    try:
        from trn_agent_boot.trn_boot import _ntff_profile_via_ctypes
        import antenv
        hook = _ntff_profile_via_ctypes("/opt/axon/libaxon_pjrt.so")
        mod = types.ModuleType("antenv.axon_hooks")
        mod.get_axon_ntff_profile_hook = lambda: hook
        mod.set_axon_ntff_profile_hook = lambda h: None
        sys.modules["antenv.axon_hooks"] = mod
        antenv.axon_hooks = mod
    except Exception:
        pass


def _regions():
    return ((0, 512), (512, 768))


def build(n_layers=N_LAYERS, zero_bias=False):
    nc = bacc.Bacc(None, target_bir_lowering=False, num_devices=8)
    L = n_layers
    W8 = FP8 if FP8_ATTN else BF16

    x_d = nc.dram_tensor("x", [P, NB_T, D], F32R, kind="ExternalInput")
    wq_d = nc.dram_tensor("wq", [L, P, NB_D * D], W8, kind="ExternalInput")
    wv_d = nc.dram_tensor("wv", [L, P, NB_D * D], W8, kind="ExternalInput")
    wo_d = nc.dram_tensor("wo", [L, P, NB_D * D], W8, kind="ExternalInput")
    w1_d = nc.dram_tensor("w1", [L, 4, P, NB_D * D], BF16, kind="ExternalInput")
    w2_d = nc.dram_tensor("w2", [L, 4, P, NB_D * D], BF16, kind="ExternalInput")
    bq_d = nc.dram_tensor("bq", [P, L, NB_D], F32, kind="ExternalInput")
    bvf_d = nc.dram_tensor("bvf", [P, L, NB_D], F32, kind="ExternalInput")
    b1_d = nc.dram_tensor("b1", [P, L, NB_FF], F32, kind="ExternalInput")
    bvr_d = nc.dram_tensor("bv_row", [1, L, D], F32, kind="ExternalInput")
    bor_d = nc.dram_tensor("bo_row", [1, L, D], BF16, kind="ExternalInput")
    b2r_d = nc.dram_tensor("b2_row", [1, L, D], BF16, kind="ExternalInput")
    idbf_d = nc.dram_tensor("identbf", [P, P], BF16, kind="ExternalInput")
    ones_d = nc.dram_tensor("ones1", [1, P], BF16, kind="ExternalInput")
    sel2_d = nc.dram_tensor("sel2", [2, P], BF16, kind="ExternalInput")
    out_d = nc.dram_tensor("out", [P, NB_T, D], F32R, kind="ExternalOutput")

    with tile.TileContext(nc) as tc:
        with (
            tc.tile_pool(name="state", bufs=1) as st,
            tc.tile_pool(name="stream", bufs=2) as sp,
            tc.tile_pool(name="acts", bufs=1) as ap,
            tc.tile_pool(name="wpool", bufs=2) as wp,
            tc.tile_pool(name="psA", bufs=2, space="PSUM") as psA,
            tc.tile_pool(name="psB", bufs=4, space="PSUM") as psB,
            tc.tile_pool(name="dram", bufs=2, space="DRAM") as dp,
        ):
            # ---- initial stream first: x gates layer 0 ----
            x_t = sp.tile([P, NB_T, D], F32R, tag="stream", name="x0")
            nc.sync.dma_start(x_t[:], x_d[:])

            # ---- constants ----
            idbf = st.tile([P, P], BF16)
            ones1 = st.tile([1, P], BF16)
            sel_a = st.tile([1, P], BF16)
            sel_b = st.tile([1, P], BF16)
            bq_all = st.tile([P, L, NB_D], F32)
            bvf_all = st.tile([P, L, NB_D], F32)
            b1_all = st.tile([P, L, NB_FF], F32)
            eps_t = st.tile([P, 1], F32)
            q_par = [st.tile([P, NB_D, T_OWN], A8, name=f"qpar{p}") for p in range(2)]
            if not SCORES_RT:
                nc.vector.memset(q_par[0][:], 0.0)
                nc.vector.memset(q_par[1][:], 0.0)
            nc.sync.dma_start(idbf[:], idbf_d[:])
            nc.sync.dma_start(ones1[:], ones_d[:])
            nc.sync.dma_start(sel_a[:], sel2_d[0:1, :])
            nc.sync.dma_start(sel_b[:], sel2_d[1:2, :])
            nc.sync.dma_start(bq_all[:], bq_d[:])
            nc.sync.dma_start(bvf_all[:], bvf_d[:])
            nc.sync.dma_start(b1_all[:], b1_d[:])
            nc.vector.memset(eps_t[:], LN_EPS)

            def ln_stats(in_ap, mv_ap):
                """bn stats for one [128, 768] block -> mv_ap [128, 2]."""
                t = ap.tile([P, 2, 6], F32, tag="lnscratch", bufs=3, name="lnt")
                xg = in_ap.rearrange("p (g d) -> p g d", g=2)
                for g in range(2):
                    nc.vector.bn_stats(t[:, g, :], xg[:, g, :])
                nc.vector.bn_aggr(mv_ap, t[:])

            def ln_rstd(mv4, nblk, name):
                """rstd [128, nblk] = (var + eps) ** -0.5, batched.

                Sqrt on ACT (one table) + fast reciprocal on DVE keeps the
                critical path to a single activation-table dependency."""
                std = ap.tile([P, nblk], F32, tag="lnt2", bufs=2, name=f"lns{name}")
                rstd = ap.tile([P, nblk], F32, tag="rstd", bufs=2, name=f"rstd{name}")
                nc.scalar.activation(
                    std[:], mv4[:, :, 1:2].rearrange("p a b -> p (a b)"),
                    AF.Sqrt, bias=eps_t[:], scale=1.0,
                )
                nc.vector.reciprocal_approx_fast(rstd[:], std[:])
                return rstd

            def sqrt_prefetch(name):
                """Dummy Sqrt so the table load lands while ACT is idle."""
                t = ap.tile([P, 1], F32, tag="sqpf", bufs=2, name=name)
                nc.scalar.activation(t[:], eps_t[:], AF.Sqrt, scale=1.0)

            def ln_apply(out_ap, in_ap, mv_ap, rstd_ap):
                nc.vector.tensor_scalar(
                    out=out_ap, in0=in_ap, scalar1=mv_ap[:, 0:1], scalar2=rstd_ap,
                    op0=OP.subtract, op1=OP.mult,
                )

            def transpose_tm_to_fm(h_tm, dtype, tag, name):
                """[128, 4, 768] token-major bf16 -> [128, 6, 512] feature-major.

                Per-token-block (tb) granularity so each block transposes as
                soon as its LN apply lands, pipelining into the previous
                phase instead of serializing at the layer boundary."""
                h_fm = ap.tile([P, NB_D, T_OWN], dtype, tag=tag, bufs=1, name=name)
                for tb in range(NB_T):
                    trp = psB.tile([P, NB_D * P], BF16, tag="psB",
                                   name=f"{name}_tr{tb}")
                    for db in range(NB_D):
                        nc.tensor.transpose(
                            trp[:, db * P:(db + 1) * P],
                            h_tm[:, tb, db * P:(db + 1) * P], idbf[:],
                        )
                    nc.vector.tensor_copy(
                        h_fm[:, :, tb * P:(tb + 1) * P],
                        trp[:].rearrange("p (d c) -> p d c", c=P),
                    )
                return h_fm

            for l in range(L):
                # ---- weights for this layer ----
                wq = wp.tile([P, NB_D, D], W8, tag="w8", bufs=4, name=f"wq{l}")
                nc.sync.dma_start(wq[:], wq_d[l].rearrange("p (k n) -> p k n", n=D))
                wv = wp.tile([P, NB_D, D], W8, tag="w8", bufs=4, name=f"wv{l}")
                nc.sync.dma_start(wv[:], wv_d[l].rearrange("p (k n) -> p k n", n=D))
                if not zero_bias:
                    bv_row = ap.tile([1, D], F32, tag="bvrow", bufs=2, name=f"bvr{l}")
                    nc.sync.dma_start(bv_row[:], bvr_d[:, l, :])
                    bo_row = ap.tile([1, D], BF16, tag="borow", bufs=2, name=f"bor{l}")
                    nc.sync.dma_start(bo_row[:], bor_d[:, l, :])
                    b2_row = ap.tile([1, D], BF16, tag="b2row", bufs=2, name=f"b2r{l}")
                    nc.sync.dma_start(b2_row[:], b2r_d[:, l, :])

                # ---- LN1 (batched rstd) + transpose ----
                with nc.named_scope(f"L{l:02d}_a_ln1"):
                    h_tm = ap.tile([P, NB_T, D], BF16, tag="h_tm", bufs=1,
                                   name=f"h1tm{l}")
                    mv4 = ap.tile([P, NB_T, 2], F32, tag="mv4", bufs=2,
                                  name=f"mv1_{l}")
                    for tb in range(NB_T):
                        ln_stats(x_t[:, tb, :], mv4[:, tb, :])
                    rstd = ln_rstd(mv4, NB_T, f"a{l}")
                    for tb in range(NB_T):
                        ln_apply(h_tm[:, tb, :], x_t[:, tb, :], mv4[:, tb, :],
                                 rstd[:, tb:tb + 1])
                    h1_fm = transpose_tm_to_fm(h_tm, A8, "h1_fm", f"h1fm{l}")

                def mm_pairs(out_ap, lhs_fn, rhs_fn, nk, extra_mm=None):
                    """Contraction over nk 128-blocks; fp8 uses DoubleRow pairs."""
                    if FP8_ATTN:
                        npair = nk // 2
                        for k in range(npair):
                            nc.tensor.matmul(
                                out_ap, lhs_fn(2 * k, 2), rhs_fn(2 * k, 2),
                                start=(k == 0),
                                stop=(k == npair - 1 and extra_mm is None),
                                perf_mode=DR,
                            )
                    else:
                        for k in range(nk):
                            nc.tensor.matmul(
                                out_ap, lhs_fn(k, 1), rhs_fn(k, 1),
                                start=(k == 0),
                                stop=(k == nk - 1 and extra_mm is None),
                            )
                    if extra_mm is not None:
                        extra_mm()

                # ---- V token-major (+bias) -> send buffer ----
                nc.enter_named_scope(f"L{l:02d}_b_v", False)
                if not zero_bias:
                    bv_bc = ap.tile([P, D], F32, tag="bv_bc", bufs=1, name=f"bvbc{l}")
                    nc.gpsimd.partition_broadcast(bv_bc[:], bv_row[:])
                v_send = ap.tile([P, NB_T, D], A8, tag="vsend", bufs=1,
                                 name=f"vsend{l}")
                for tb in range(NB_T):
                    vp = psA.tile([P, D], F32, tag="psA", name=f"vps{l}_{tb}")
                    for n0, n1 in _regions():
                        mm_pairs(
                            vp[:, n0:n1],
                            lambda k, w: h1_fm[:, k:k + w, tb * P:(tb + 1) * P],
                            lambda k, w: wv[:, k:k + w, n0:n1],
                            NB_D,
                        )
                    if zero_bias:
                        nc.vector.tensor_copy(v_send[:, tb, :], vp[:])
                    else:
                        nc.vector.tensor_tensor(
                            v_send[:, tb, :], vp[:], bv_bc[:], OP.add
                        )

                # own half of v_fm directly as feature-major matmuls (no
                # transposes needed; also what the peer receives pre-made)
                v_fm = ap.tile([P, NB_D, NB_K * P], A8, tag="v_fm", bufs=1,
                               name=f"vfm{l}")
                vsend2_dr = dp.tile([NB_D, P, T_OWN], A8, tag="vsend2_d",
                                    name=f"vs2dr{l}")
                for db in range(NB_D):
                    vq = psB.tile([P, T_OWN], F32, tag="psB", name=f"vfps{l}_{db}")
                    mm_pairs(
                        vq[:],
                        lambda k, w: wv[:, k:k + w, db * P:(db + 1) * P],
                        lambda k, w: h1_fm[:, k:k + w, :],
                        NB_D,
                    )
                    nc.vector.tensor_scalar(
                        out=v_fm[:, db, 0:T_OWN], in0=vq[:],
                        scalar1=bvf_all[:, l, db:db + 1], scalar2=None,
                        op0=OP.add,
                    )
                    nc.sync.dma_start(vsend2_dr[db], v_fm[:, db, 0:T_OWN])

                nc.leave_named_scope(f"L{l:02d}_b_v", None, False)
                nc.enter_named_scope(f"L{l:02d}_c_ag", False)
                vsend_dr = dp.tile([NB_T, P, D], A8, tag="vsend_d", name=f"vsdr{l}")
                for tb in range(NB_T):
                    nc.sync.dma_start(vsend_dr[tb], v_send[:, tb, :])
                vrecv_dr = dp.tile([NB_K, P, D], A8, tag="vrecv_d", name=f"vrdr{l}")
                vrecv2_dr = dp.tile([2 * NB_D, P, T_OWN], A8, tag="vrecv2_d",
                                    name=f"vr2dr{l}")
                nc.gpsimd.collective_compute(
                    "AllGather", OP.bypass, replica_groups=REPLICA_GROUPS,
                    ins=[vsend_dr[:]], outs=[vrecv_dr[:]],
                )
                nc.gpsimd.collective_compute(
                    "AllGather", OP.bypass, replica_groups=REPLICA_GROUPS,
                    ins=[vsend2_dr[:]], outs=[vrecv2_dr[:]],
                )
                nc.leave_named_scope(f"L{l:02d}_c_ag", None, False)
                nc.enter_named_scope(f"L{l:02d}_d_q", False)
                # ---- Q (overlaps the AllGather) ----
                for m in range(NB_D):
                    qp = psB.tile([P, T_OWN], F32, tag="psB", name=f"qps{l}_{m}")
                    mm_pairs(
                        qp[:],
                        lambda k, w: wq[:, k:k + w, m * P:(m + 1) * P],
                        lambda k, w: h1_fm[:, k:k + w, :],
                        NB_D,
                    )
                    for hh in range(2):
                        r0 = 64 * hh
                        nc.vector.tensor_scalar(
                            out=q_par[hh][r0:r0 + 64, m, :], in0=qp[r0:r0 + 64, :],
                            scalar1=bq_all[r0:r0 + 64, l, m:m + 1], scalar2=None,
                            op0=OP.add,
                        )

                # prefetch Wo while attention runs
                wo = wp.tile([P, NB_D, D], W8, tag="w8", bufs=4, name=f"wo{l}")
                nc.sync.dma_start(wo[:], wo_d[l].rearrange("p (k n) -> p k n", n=D))

                nc.leave_named_scope(f"L{l:02d}_d_q", None, False)
                nc.enter_named_scope(f"L{l:02d}_e_vrecv", False)
                # ---- receive V: augmented token-major (A8) + feature-major bf16 ----
                # own-relative key order: kb 0-3 = own half (local data),
                # kb 4-7 = peer half (dynamic shard pick from vrecv)
                # row stride padded to 784 (16-aligned) for DoubleRow LDWEIGHTS
                v_aug_full = ap.tile([P, NB_K, 784], A8, tag="v_aug", bufs=1,
                                     name=f"vaug{l}")
                v_aug = v_aug_full[:, :, 0:H * 65]
                va4 = v_aug.rearrange("p k (h c) -> p k h c", c=65)
                for kb in range(NB_K):
                    nc.vector.memset(va4[:, kb, :, 64:65], 1.0)
                for tb in range(NB_T):
                    nc.vector.tensor_copy(
                        va4[:, tb, :, 0:64],
                        v_send[:, tb, :].rearrange("p (h c) -> p h c", c=64),
                    )
                peer_off = (1 - (nc.gpsimd.partition_id() % 2)) * NB_T
                vr4 = vrecv_dr.rearrange("k p (h c) -> k p h c", c=64)
                for tb in range(NB_T):
                    nc.gpsimd.dma_start(
                        va4[:, 4 + tb, :, 0:64], vr4[ds(peer_off + tb, 1)],
                    )
                # peer half of v_fm arrives pre-transposed from the AllGather
                peer_off2 = (1 - (nc.gpsimd.partition_id() % 2)) * NB_D
                for db in range(NB_D):
                    nc.gpsimd.dma_start(
                        v_fm[:, db, T_OWN:], vrecv2_dr[ds(peer_off2 + db, 1)],
                    )
                nc.leave_named_scope(f"L{l:02d}_e_vrecv", None, False)
                nc.enter_named_scope(f"L{l:02d}_f_attn", False)
                # ---- attention: flat software pipeline over (head-pair, kb-pair)
                # steps; ctx lags scores by one step; db 0-1 own-half steps overlap
                # the AllGather ----
                ctx_n = ap.tile([P, NB_D, T_OWN], A8, tag="ctx_n", bufs=1,
                                name=f"ctxn{l}")
                OWN_PAIRS = [(0, 1), (2, 3)]
                PEER_PAIRS = [(4, 5), (6, 7)]

                ctxps = {}
                ex_own = {}
                pending = None  # (db, pair, ex, is_first_peer, is_last)

                def score_mms(db, pair, spv):
                    for o in range(2):
                        kb = pair[o]
                        for hh in range(2):
                            if SCORES_RT:
                                r0 = 64 * hh
                                nc.tensor.matmul(
                                    spv[hh][:, o * 512:(o + 1) * 512],
                                    v_fm[r0:r0 + 64, db, kb * P:(kb + 1) * P],
                                    q_par[hh][r0:r0 + 64, db, :],
                                    start=True, stop=True,
                                )
                            else:
                                nc.tensor.matmul(
                                    spv[hh][:, o * 512:(o + 1) * 512],
                                    v_fm[:, db, kb * P:(kb + 1) * P],
                                    q_par[hh][:, db, :],
                                    start=True, stop=True,
                                )

                def exp_tiles(l, db, pair, spv, tag, bufs):
                    ex = [
                        ap.tile([P, 2, 512], A8, tag=tag, bufs=bufs,
                                name=f"ex{l}_{db}_{pair[0]}_{hh}")
                        for hh in range(2)
                    ]
                    for hh in range(2):
                        nc.scalar.activation(
                            ex[hh].rearrange("p a b -> p (a b)"), spv[hh][:],
                            AF.Exp, scale=SCALE / (WS * WS),
                        )
                    return ex

                def ctx_mm(db, pair, ex, start, stop):
                    hpair = (2 * db, 2 * db + 1)
                    for hh in range(2):
                        h = hpair[hh]
                        if FP8_ATTN:
                            nc.tensor.matmul(
                                ctxps[db][hh][:],
                                v_aug[:, pair[0]:pair[0] + 2, 65 * h:65 * h + 65],
                                ex[hh][:],
                                start=start, stop=stop,
                                perf_mode=DR,
                            )
                        else:
                            for o in range(2):
                                nc.tensor.matmul(
                                    ctxps[db][hh][:],
                                    v_aug[:, pair[o], 65 * h:65 * h + 65],
                                    ex[hh][:, o, :],
                                    start=(start and o == 0), stop=(stop and o == 1),
                                )

                def flush_ctx():
                    nonlocal pending
                    if pending is None:
                        return
                    db, pair, ex, first, last = pending
                    if first:
                        ctxps[db] = [
                            psB.tile([65, T_OWN], F32, tag="psB",
                                     name=f"ctxp{l}_{2 * db + hh}")
                            for hh in range(2)
                        ]
                        for j, opair in enumerate(OWN_PAIRS):
                            ctx_mm(db, opair, ex_own.pop((db, opair)),
                                   start=(j == 0), stop=False)
                    ctx_mm(db, pair, ex, start=False, stop=last)
                    if last:
                        # Z rows -> bf16 sbuf, broadcast both heads into one
                        # [128,512] psum via two accumulating K=1 outer products
                        # on the PE, one 128-lane reciprocal, then one DVE
                        # multiply per head.
                        zrow = [
                            ap.tile([1, T_OWN], BF16, tag="zrow", bufs=4,
                                    name=f"zr{l}_{db}_{hh}")
                            for hh in range(2)
                        ]
                        for hh in range(2):
                            nc.vector.tensor_copy(
                                zrow[hh][:], ctxps[db][hh][64:65, :]
                            )
                        zbcp = psA.tile([P, T_OWN], F32, tag="psA",
                                        name=f"zbp{l}_{db}")
                        nc.tensor.matmul(
                            zbcp[:], sel_a[:], zrow[0][:],
                            start=True, stop=False,
                        )
                        nc.tensor.matmul(
                            zbcp[:], sel_b[:], zrow[1][:],
                            start=False, stop=True,
                        )
                        zbci = ap.tile([P, T_OWN], F32, tag="zbc", bufs=2,
                                       name=f"zb{l}_{db}")
                        nc.vector.reciprocal_approx_fast(zbci[:], zbcp[:])
                        for hh in range(2):
                            r0 = 64 * hh
                            nc.vector.tensor_tensor(
                                ctx_n[r0:r0 + 64, db, :], ctxps[db][hh][0:64, :],
                                zbci[r0:r0 + 64, :], OP.mult,
                            )
                        del ctxps[db]
                    pending = None

                # phase 1: own-half scores+exp for every head pair, buffered.
                # Runs during the AllGather window (needs only v_fm own half
                # and q_par), keeping the PE warm while the collective runs.
                for db in range(NB_D):
                    for pair in OWN_PAIRS:
                        spv = [
                            psA.tile([P, 1024], F32, tag="psA",
                                     name=f"sc{l}_{db}_{pair[0]}_{hh}")
                            for hh in range(2)
                        ]
                        score_mms(db, pair, spv)
                        ex_own[(db, pair)] = exp_tiles(l, db, pair, spv,
                                                       "expOwn", 24)

                # phase 2: peer-half scores+exp pipelined with ctx accumulation
                # (own-half ctx MMs issue from the buffered exps).
                for db in range(NB_D):
                    for pi, pair in enumerate(PEER_PAIRS):
                        spv = [
                            psA.tile([P, 1024], F32, tag="psA",
                                     name=f"sc{l}_{db}_{pair[0]}_{hh}")
                            for hh in range(2)
                        ]
                        score_mms(db, pair, spv)
                        flush_ctx()
                        ex = exp_tiles(l, db, pair, spv, "expT", 5)
                        pending = (db, pair, ex, pi == 0,
                                   pi == len(PEER_PAIRS) - 1)
                flush_ctx()
                nc.leave_named_scope(f"L{l:02d}_f_attn", None, False)
                nc.enter_named_scope(f"L{l:02d}_g_wo", False)
                # ---- Wo + residual; LN2 stats per block as it completes ----
                sqrt_prefetch(f"sqp2_{l}")
                skip = sp.tile([P, NB_T, D], F32R, tag="stream", name=f"skip{l}")
                mv4b = ap.tile([P, NB_T, 2], F32, tag="mv4", bufs=2, name=f"mv2_{l}")
                for lb in range(NB_T):
                    wps = psA.tile([P, D], F32, tag="psA", name=f"wops{l}_{lb}")
                    for n0, n1 in _regions():
                        mm_pairs(
                            wps[:, n0:n1],
                            lambda k, w: ctx_n[:, k:k + w, lb * P:(lb + 1) * P],
                            lambda k, w: wo[:, k:k + w, n0:n1],
                            NB_D,
                            extra_mm=None if zero_bias else (
                                lambda n0=n0, n1=n1: nc.tensor.matmul(
                                    wps[:, n0:n1], ones1[:], bo_row[:, n0:n1],
                                    start=False, stop=True,
                                )
                            ),
                        )
                    if FP8_ATTN:
                        nc.vector.scalar_tensor_tensor(
                            out=skip[:, lb, :], in0=wps[:], scalar=1.0 / (WS * WS),
                            in1=x_t[:, lb, :], op0=OP.mult, op1=OP.add,
                        )
                    else:
                        nc.vector.tensor_tensor(
                            skip[:, lb, :], x_t[:, lb, :], wps[:], OP.add
                        )
                    ln_stats(skip[:, lb, :], mv4b[:, lb, :])
                nc.leave_named_scope(f"L{l:02d}_g_wo", None, False)
                nc.enter_named_scope(f"L{l:02d}_h_ln2", False)
                # ---- LN2 apply + transpose ----
                h_tm2 = ap.tile([P, NB_T, D], BF16, tag="h_tm", bufs=1,
                                name=f"h2tm{l}")
                rstd2 = ln_rstd(mv4b, NB_T, f"b{l}")
                for lb in range(NB_T):
                    ln_apply(h_tm2[:, lb, :], skip[:, lb, :], mv4b[:, lb, :],
                             rstd2[:, lb:lb + 1])
                h2_fm = transpose_tm_to_fm(h_tm2, BF16, "h2_fm", f"h2fm{l}")
                nc.leave_named_scope(f"L{l:02d}_h_ln2", None, False)
                nc.enter_named_scope(f"L{l:02d}_i_ff", False)
                # ---- FFN: FF1 (all 24 hidden blocks) then FF2 in PSUM (bf16) ----
                g_all = ap.tile([P, NB_FF, T_OWN], BF16, tag="g", bufs=1, name=f"g{l}")
                w2cs = []
                for ck in range(4):
                    w1c = wp.tile([P, NB_D, D], BF16, tag="wbf", bufs=5,
                                  name=f"w1c{l}_{ck}")
                    nc.sync.dma_start(w1c[:], w1_d[l, ck].rearrange("p (k n) -> p k n", n=D))
                    w2c = wp.tile([P, NB_D, D], BF16, tag="wbf", bufs=5,
                                  name=f"w2c{l}_{ck}")
                    nc.sync.dma_start(w2c[:], w2_d[l, ck].rearrange("p (k n) -> p k n", n=D))
                    w2cs.append(w2c)
                    for mm in range(NB_D):
                        fp = psB.tile([P, T_OWN], F32, tag="psB", name=f"f1ps{l}_{ck}_{mm}")
                        for kb in range(NB_D):
                            nc.tensor.matmul(
                                fp[:], w1c[:, kb, mm * P:(mm + 1) * P], h2_fm[:, kb, :],
                                start=(kb == 0), stop=(kb == NB_D - 1),
                            )
                        nc.scalar.activation(
                            g_all[:, 6 * ck + mm, :], fp[:], AF.Gelu,
                            bias=b1_all[:, l, 6 * ck + mm:6 * ck + mm + 1], scale=1.0,
                        )
                sqrt_prefetch(f"sqp1_{l}")
                for half in range(2):
                    f2s = []
                    for lb in (2 * half, 2 * half + 1):
                        f2 = psA.tile([P, D], F32, tag="psA", name=f"f2ps{l}_{lb}")
                        f2s.append(f2)
                        for n0, n1 in _regions():
                            for ck in range(4):
                                for mm in range(NB_D):
                                    last = zero_bias and ck == 3 and mm == NB_D - 1
                                    nc.tensor.matmul(
                                        f2[:, n0:n1],
                                        g_all[:, 6 * ck + mm, lb * P:(lb + 1) * P],
                                        w2cs[ck][:, mm, n0:n1],
                                        start=(ck == 0 and mm == 0), stop=last,
                                    )
                            if not zero_bias:
                                nc.tensor.matmul(
                                    f2[:, n0:n1], ones1[:], b2_row[:, n0:n1],
                                    start=False, stop=True,
                                )
                    for i, lb in enumerate((2 * half, 2 * half + 1)):
                        nc.vector.tensor_tensor(
                            skip[:, lb, :], skip[:, lb, :], f2s[i][:], OP.add,
                        )
                nc.leave_named_scope(f"L{l:02d}_i_ff", None, False)
                x_t = skip

            nc.sync.dma_start(out_d[:], x_t[:])
    nc.compile()
    return nc


def _preprocess(inputs, n_layers):
    """Fold LN affine into projections; lay out weights for tile DMA."""
    f32 = np.float32
    L = n_layers
    Wq = np.asarray(inputs["Wq"], f32)[:L]
    Wv = np.asarray(inputs["Wv"], f32)[:L]
    Wo = np.asarray(inputs["Wo"], f32)[:L]
    W1 = np.asarray(inputs["W1"], f32)[:L]
    W2 = np.asarray(inputs["W2"], f32)[:L]
    g1 = np.asarray(inputs["ln1_g"], f32)[:L]
    b1ln = np.asarray(inputs["ln1_b"], f32)[:L]
    g2 = np.asarray(inputs["ln2_g"], f32)[:L]
    b2ln = np.asarray(inputs["ln2_b"], f32)[:L]
    bq = np.asarray(inputs["bq"], f32)[:L]
    bv = np.asarray(inputs["bv"], f32)[:L]
    bo = np.asarray(inputs["bo"], f32)[:L]
    b1 = np.asarray(inputs["b1"], f32)[:L]
    b2 = np.asarray(inputs["b2"], f32)[:L]

    Wq_eff = g1[:, :, None] * Wq
    bq_eff = bq + np.einsum("ld,ldo->lo", b1ln, Wq)
    Wv_eff = g1[:, :, None] * Wv
    bv_eff = bv + np.einsum("ld,ldo->lo", b1ln, Wv)
    W1_eff = g2[:, :, None] * W1
    b1_eff = b1 + np.einsum("ld,ldo->lo", b2ln, W1)

    def fm_weight(W):  # [L, D, D] -> [L, 128, 6*768] with [p, k, n]
        return np.ascontiguousarray(
            W.reshape(L, NB_D, P, D).transpose(0, 2, 1, 3).reshape(L, P, NB_D * D)
        )

    bf = ml_dtypes.bfloat16
    f8 = ml_dtypes.float8_e4m3
    w8 = f8 if FP8_ATTN else bf
    wq_h = fm_weight(Wq_eff * WS).astype(w8)
    wv_h = fm_weight(Wv_eff * WS).astype(w8)
    wo_h = fm_weight(Wo * WS).astype(w8)
    w1_h = np.ascontiguousarray(
        W1_eff.reshape(L, NB_D, P, 4, D).transpose(0, 3, 2, 1, 4).reshape(L, 4, P, NB_D * D)
    ).astype(bf)
    w2_h = np.ascontiguousarray(
        W2.reshape(L, 4, NB_D, P, D).transpose(0, 1, 3, 2, 4).reshape(L, 4, P, NB_D * D)
    ).astype(bf)
    bq_h = np.ascontiguousarray((bq_eff * WS).reshape(L, NB_D, P).transpose(2, 0, 1))
    bvf_h = np.ascontiguousarray((bv_eff * WS).reshape(L, NB_D, P).transpose(2, 0, 1))
    b1_h = np.ascontiguousarray(b1_eff.reshape(L, NB_FF, P).transpose(2, 0, 1))

    sel2 = np.zeros((2, P), f32)
    sel2[0, 0:64] = 1.0
    sel2[1, 64:128] = 1.0
    sel2 = sel2.astype(bf)

    return {
        "wq": wq_h, "wv": wv_h, "wo": wo_h, "w1": w1_h, "w2": w2_h,
        "bq": bq_h, "bvf": bvf_h, "b1": b1_h,
        "bv_row": np.ascontiguousarray(bv_eff[None] * WS),
        "bo_row": np.ascontiguousarray(bo[None] * WS * WS).astype(bf),
        "b2_row": np.ascontiguousarray(b2[None]).astype(bf),
        "identbf": np.eye(P).astype(bf),
        "ones1": np.ones((1, P)).astype(bf),
        "sel2": sel2,
    }


def kernel(**inputs) -> np.ndarray:
    n_layers = N_LAYERS
    zero_bias = not any(
        np.any(np.asarray(inputs[k])) for k in ("bv", "bo", "b2", "ln1_b")
    )
    key = ("nc", n_layers, zero_bias)
    if key not in _cached:
        _cached[key] = build(n_layers, zero_bias)
    nc = _cached[key]

    shared = _preprocess(inputs, n_layers)
    x = np.asarray(inputs["x"], np.float32)  # [4, 1024, 768]
    B, T, _ = x.shape

    in_maps = []
    for c in range(8):
        b, half = c // 2, c % 2
        x_own = x[b, half * T_OWN:(half + 1) * T_OWN]          # [512, 768]
        x_tile = np.ascontiguousarray(
            x_own.reshape(NB_T, P, D).transpose(1, 0, 2)        # [128, 4, 768]
        )
        in_maps.append({**shared, "x": x_tile})

    trace = bool(int(os.environ.get("KERNEL_TRACE", "0")))
    if trace:
        _register_ntff_hook()
    res = run_bass_kernel_spmd(nc, in_maps, core_ids=list(range(8)), trace=trace)
    global _last_results
    _last_results = res

    out = np.empty((B, T, D), dtype=np.float32)
    for c in range(8):
        b, half = c // 2, c % 2
        o = res.results[c]["out"]                               # [128, 4, 768]
        out[b, half * T_OWN:(half + 1) * T_OWN] = (
            o.transpose(1, 0, 2).reshape(T_OWN, D)
        )
    return out


# revision 54
# speedup vs baseline: 1.2801x; 1.0181x over previous
"""Trainium2 Bass kernel for nn_Encoder_506806141403.

12-layer transformer encoder (D=768, H=12, FF=3072) with the quirk that
attention scores use Q vs V (no K projection) and scale by D**-0.5.

Sharding: 8 cores = 4 batch elements x 2 sequence halves. Each core owns
512 query rows of one batch element. Per layer, each core computes its half's
V projection (token-major, bf16) and the pair exchanges halves via a 2-rank
AllGather, so every core holds all 1024 keys/values of its batch element.
Everything else (LN, Q, scores, softmax, ctx, Wo, FFN) is computed per-core
on its own 512 rows.

v2 performance changes vs. baseline:
- Attention-path GEMMs (V, Q, Wo, probs@V) run in fp8e4 DoubleRow perf mode
  (2x PE throughput; weights host-scaled x64 to stay in fp8 normal range,
  descale folded into the softmax scale / fused evacuation multiplies).
  FFN + scores stay bf16: fp8 there would exceed the 2e-2 error budget.
- Softmax Z: reciprocal_approx_fast on [1,512] rows + a K=2 outer-product
  matmul broadcast (replaces exact DVE reciprocal 3.3us/head + gpsimd
  partition_broadcast 1us/head).
- LayerNorm rstd (exp(-0.5*ln(var+eps))) computed once per 4-block phase on
  [128,4] stats instead of per block, eliminating ACT table thrash.
- LN transposes in bf16 (1.0 cyc/row on PE vs 1.5 for f32r).
- Optional row-tiled scores (64x128 PE tiling, two heads concurrently).
"""
import os
import sys

sys.path.insert(0, "/opt/trn_rl_repo")

import numpy as np
import ml_dtypes

import concourse.bass as bass
from concourse.bass import ds
from concourse import bacc
import concourse.tile as tile
from concourse import mybir
from concourse.bass_utils import run_bass_kernel_spmd

P = 128
D = 768
H = 12
DH = 64
FF = 3072
NB_D = 6          # D / P
NB_T = 4          # own tokens 512 / P
NB_K = 8          # full tokens 1024 / P
NB_FF = 24        # FF / P
T_OWN = 512
SCALE = float(D) ** -0.5
LN_EPS = 1e-5
N_LAYERS = int(os.environ.get("KERNEL_N_LAYERS", "12"))
FP8_ATTN = bool(int(os.environ.get("KERNEL_FP8_ATTN", "1")))
SCORES_RT = bool(int(os.environ.get("KERNEL_SCORES_RT", "0")))
WS = 64.0 if FP8_ATTN else 1.0   # host-side weight scale for fp8 range

F32 = mybir.dt.float32
F32R = mybir.dt.float32r
BF16 = mybir.dt.bfloat16
FP8 = mybir.dt.float8e4
AF = mybir.ActivationFunctionType
OP = mybir.AluOpType
DR = mybir.MatmulPerfMode.DoubleRow

A8 = FP8 if FP8_ATTN else BF16   # attention-path activation dtype

REPLICA_GROUPS = [[0, 1], [2, 3], [4, 5], [6, 7]]

_cached = {}
_last_results = None


def _register_ntff_hook():
    """Register the axon NTFF profile hook (for trace=True exec timing)."""
    import types
    try:
        import antenv.axon_hooks  # noqa: F401
        return
    except ImportError:
        pass
    try:
        from trn_agent_boot.trn_boot import _ntff_profile_via_ctypes
        import antenv
        hook = _ntff_profile_via_ctypes("/opt/axon/libaxon_pjrt.so")
        mod = types.ModuleType("antenv.axon_hooks")
        mod.get_axon_ntff_profile_hook = lambda: hook
        mod.set_axon_ntff_profile_hook = lambda h: None
        sys.modules["antenv.axon_hooks"] = mod
        antenv.axon_hooks = mod
    except Exception:
        pass


def _regions():
    return ((0, 512), (512, 768))


def build(n_layers=N_LAYERS, zero_bias=False):
    nc = bacc.Bacc(None, target_bir_lowering=False, num_devices=8)
    L = n_layers
    W8 = FP8 if FP8_ATTN else BF16

    x_d = nc.dram_tensor("x", [P, NB_T, D], F32R, kind="ExternalInput")
    wq_d = nc.dram_tensor("wq", [L, P, NB_D * D], W8, kind="ExternalInput")
    wv_d = nc.dram_tensor("wv", [L, P, NB_D * D], W8, kind="ExternalInput")
    wo_d = nc.dram_tensor("wo", [L, P, NB_D * D], W8, kind="ExternalInput")
    w1_d = nc.dram_tensor("w1", [L, 4, P, NB_D * D], BF16, kind="ExternalInput")
    w2_d = nc.dram_tensor("w2", [L, 4, P, NB_D * D], BF16, kind="ExternalInput")
    bq_d = nc.dram_tensor("bq", [P, L, NB_D], F32, kind="ExternalInput")
    bvf_d = nc.dram_tensor("bvf", [P, L, NB_D], F32, kind="ExternalInput")
    b1_d = nc.dram_tensor("b1", [P, L, NB_FF], F32, kind="ExternalInput")
    bvr_d = nc.dram_tensor("bv_row", [1, L, D], F32, kind="ExternalInput")
    bor_d = nc.dram_tensor("bo_row", [1, L, D], BF16, kind="ExternalInput")
    b2r_d = nc.dram_tensor("b2_row", [1, L, D], BF16, kind="ExternalInput")
    idbf_d = nc.dram_tensor("identbf", [P, P], BF16, kind="ExternalInput")
    ones_d = nc.dram_tensor("ones1", [1, P], BF16, kind="ExternalInput")
    sel2_d = nc.dram_tensor("sel2", [2, P], BF16, kind="ExternalInput")
    out_d = nc.dram_tensor("out", [P, NB_T, D], F32R, kind="ExternalOutput")

    with tile.TileContext(nc) as tc:
        with (
            tc.tile_pool(name="state", bufs=1) as st,
            tc.tile_pool(name="stream", bufs=2) as sp,
            tc.tile_pool(name="acts", bufs=1) as ap,
            tc.tile_pool(name="wpool", bufs=2) as wp,
            tc.tile_pool(name="psA", bufs=2, space="PSUM") as psA,
            tc.tile_pool(name="psB", bufs=4, space="PSUM") as psB,
            tc.tile_pool(name="dram", bufs=2, space="DRAM") as dp,
        ):
            # ---- initial stream first: x gates layer 0 ----
            x_t = sp.tile([P, NB_T, D], F32R, tag="stream", name="x0")
            nc.sync.dma_start(x_t[:], x_d[:])

            # ---- constants ----
            idbf = st.tile([P, P], BF16)
            ones1 = st.tile([1, P], BF16)
            sel_a = st.tile([1, P], BF16)
            sel_b = st.tile([1, P], BF16)
            bq_all = st.tile([P, L, NB_D], F32)
            bvf_all = st.tile([P, L, NB_D], F32)
            b1_all = st.tile([P, L, NB_FF], F32)
            eps_t = st.tile([P, 1], F32)
            q_par = [st.tile([P, NB_D, T_OWN], A8, name=f"qpar{p}") for p in range(2)]
            if not SCORES_RT:
                nc.vector.memset(q_par[0][:], 0.0)
                nc.vector.memset(q_par[1][:], 0.0)
            nc.sync.dma_start(idbf[:], idbf_d[:])
            nc.sync.dma_start(ones1[:], ones_d[:])
            nc.sync.dma_start(sel_a[:], sel2_d[0:1, :])
            nc.sync.dma_start(sel_b[:], sel2_d[1:2, :])
            nc.sync.dma_start(bq_all[:], bq_d[:])
            nc.sync.dma_start(bvf_all[:], bvf_d[:])
            nc.sync.dma_start(b1_all[:], b1_d[:])
            nc.vector.memset(eps_t[:], LN_EPS)

            def ln_stats(in_ap, mv_ap):
                """bn stats for one [128, 768] block -> mv_ap [128, 2]."""
                t = ap.tile([P, 2, 6], F32, tag="lnscratch", bufs=3, name="lnt")
                xg = in_ap.rearrange("p (g d) -> p g d", g=2)
                for g in range(2):
                    nc.vector.bn_stats(t[:, g, :], xg[:, g, :])
                nc.vector.bn_aggr(mv_ap, t[:])

            def ln_rstd(mv4, nblk, name):
                """rstd [128, nblk] = (var + eps) ** -0.5, batched.

                Sqrt on ACT (one table) + fast reciprocal on DVE keeps the
                critical path to a single activation-table dependency."""
                std = ap.tile([P, nblk], F32, tag="lnt2", bufs=2, name=f"lns{name}")
                rstd = ap.tile([P, nblk], F32, tag="rstd", bufs=2, name=f"rstd{name}")
                nc.scalar.activation(
                    std[:], mv4[:, :, 1:2].rearrange("p a b -> p (a b)"),
                    AF.Sqrt, bias=eps_t[:], scale=1.0,
                )
                nc.vector.reciprocal_approx_fast(rstd[:], std[:])
                return rstd

            def sqrt_prefetch(name):
                """Dummy Sqrt so the table load lands while ACT is idle."""
                t = ap.tile([P, 1], F32, tag="sqpf", bufs=2, name=name)
                nc.scalar.activation(t[:], eps_t[:], AF.Sqrt, scale=1.0)

            def ln_apply(out_ap, in_ap, mv_ap, rstd_ap):
                nc.vector.tensor_scalar(
                    out=out_ap, in0=in_ap, scalar1=mv_ap[:, 0:1], scalar2=rstd_ap,
                    op0=OP.subtract, op1=OP.mult,
                )

            def transpose_tm_to_fm(h_tm, dtype, tag, name):
                """[128, 4, 768] token-major bf16 -> [128, 6, 512] feature-major.

                Per-token-block (tb) granularity so each block transposes as
                soon as its LN apply lands, pipelining into the previous
                phase instead of serializing at the layer boundary."""
                h_fm = ap.tile([P, NB_D, T_OWN], dtype, tag=tag, bufs=1, name=name)
                for tb in range(NB_T):
                    trp = psB.tile([P, NB_D * P], BF16, tag="psB",
                                   name=f"{name}_tr{tb}")
                    for db in range(NB_D):
                        nc.tensor.transpose(
                            trp[:, db * P:(db + 1) * P],
                            h_tm[:, tb, db * P:(db + 1) * P], idbf[:],
                        )
                    nc.vector.tensor_copy(
                        h_fm[:, :, tb * P:(tb + 1) * P],
                        trp[:].rearrange("p (d c) -> p d c", c=P),
                    )
                return h_fm

            for l in range(L):
                # ---- weights for this layer ----
                wq = wp.tile([P, NB_D, D], W8, tag="w8", bufs=4, name=f"wq{l}")
                nc.sync.dma_start(wq[:], wq_d[l].rearrange("p (k n) -> p k n", n=D))
                wv = wp.tile([P, NB_D, D], W8, tag="w8", bufs=4, name=f"wv{l}")
                nc.sync.dma_start(wv[:], wv_d[l].rearrange("p (k n) -> p k n", n=D))
                if not zero_bias:
                    bv_row = ap.tile([1, D], F32, tag="bvrow", bufs=2, name=f"bvr{l}")
                    nc.sync.dma_start(bv_row[:], bvr_d[:, l, :])
                    bo_row = ap.tile([1, D], BF16, tag="borow", bufs=2, name=f"bor{l}")
                    nc.sync.dma_start(bo_row[:], bor_d[:, l, :])
                    b2_row = ap.tile([1, D], BF16, tag="b2row", bufs=2, name=f"b2r{l}")
                    nc.sync.dma_start(b2_row[:], b2r_d[:, l, :])

                # ---- LN1 (batched rstd) + transpose ----
                with nc.named_scope(f"L{l:02d}_a_ln1"):
                    h_tm = ap.tile([P, NB_T, D], BF16, tag="h_tm", bufs=1,
                                   name=f"h1tm{l}")
                    mv4 = ap.tile([P, NB_T, 2], F32, tag="mv4", bufs=2,
                                  name=f"mv1_{l}")
                    for tb in range(NB_T):
                        ln_stats(x_t[:, tb, :], mv4[:, tb, :])
                    rstd = ln_rstd(mv4, NB_T, f"a{l}")
                    for tb in range(NB_T):
                        ln_apply(h_tm[:, tb, :], x_t[:, tb, :], mv4[:, tb, :],
                                 rstd[:, tb:tb + 1])
                    h1_fm = transpose_tm_to_fm(h_tm, A8, "h1_fm", f"h1fm{l}")

                def mm_pairs(out_ap, lhs_fn, rhs_fn, nk, extra_mm=None):
                    """Contraction over nk 128-blocks; fp8 uses DoubleRow pairs."""
                    if FP8_ATTN:
                        npair = nk // 2
                        for k in range(npair):
                            nc.tensor.matmul(
                                out_ap, lhs_fn(2 * k, 2), rhs_fn(2 * k, 2),
                                start=(k == 0),
                                stop=(k == npair - 1 and extra_mm is None),
                                perf_mode=DR,
                            )
                    else:
                        for k in range(nk):
                            nc.tensor.matmul(
                                out_ap, lhs_fn(k, 1), rhs_fn(k, 1),
                                start=(k == 0),
                                stop=(k == nk - 1 and extra_mm is None),
                            )
                    if extra_mm is not None:
                        extra_mm()

                # ---- V token-major (+bias) -> send buffer ----
                nc.enter_named_scope(f"L{l:02d}_b_v", False)
                if not zero_bias:
                    bv_bc = ap.tile([P, D], F32, tag="bv_bc", bufs=1, name=f"bvbc{l}")
                    nc.gpsimd.partition_broadcast(bv_bc[:], bv_row[:])
                v_send = ap.tile([P, NB_T, D], A8, tag="vsend", bufs=1,
                                 name=f"vsend{l}")
                for tb in range(NB_T):
                    vp = psA.tile([P, D], F32, tag="psA", name=f"vps{l}_{tb}")
                    for n0, n1 in _regions():
                        mm_pairs(
                            vp[:, n0:n1],
                            lambda k, w: h1_fm[:, k:k + w, tb * P:(tb + 1) * P],
                            lambda k, w: wv[:, k:k + w, n0:n1],
                            NB_D,
                        )
                    if zero_bias:
                        nc.vector.tensor_copy(v_send[:, tb, :], vp[:])
                    else:
                        nc.vector.tensor_tensor(
                            v_send[:, tb, :], vp[:], bv_bc[:], OP.add
                        )

                # own half of v_fm directly as feature-major matmuls (no
                # transposes needed; also what the peer receives pre-made)
                v_fm = ap.tile([P, NB_D, NB_K * P], A8, tag="v_fm", bufs=1,
                               name=f"vfm{l}")
                vsend2_dr = dp.tile([NB_D, P, T_OWN], A8, tag="vsend2_d",
                                    name=f"vs2dr{l}")
                for db in range(NB_D):
                    vq = psB.tile([P, T_OWN], F32, tag="psB", name=f"vfps{l}_{db}")
                    mm_pairs(
                        vq[:],
                        lambda k, w: wv[:, k:k + w, db * P:(db + 1) * P],
                        lambda k, w: h1_fm[:, k:k + w, :],
                        NB_D,
                    )
                    nc.vector.tensor_scalar(
                        out=v_fm[:, db, 0:T_OWN], in0=vq[:],
                        scalar1=bvf_all[:, l, db:db + 1], scalar2=None,
                        op0=OP.add,
                    )
                    nc.sync.dma_start(vsend2_dr[db], v_fm[:, db, 0:T_OWN])

                nc.leave_named_scope(f"L{l:02d}_b_v", None, False)
                nc.enter_named_scope(f"L{l:02d}_c_ag", False)
                vsend_dr = dp.tile([NB_T, P, D], A8, tag="vsend_d", name=f"vsdr{l}")
                for tb in range(NB_T):
                    nc.sync.dma_start(vsend_dr[tb], v_send[:, tb, :])
                vrecv_dr = dp.tile([NB_K, P, D], A8, tag="vrecv_d", name=f"vrdr{l}")
                vrecv2_dr = dp.tile([2 * NB_D, P, T_OWN], A8, tag="vrecv2_d",
                                    name=f"vr2dr{l}")
                nc.gpsimd.collective_compute(
                    "AllGather", OP.bypass, replica_groups=REPLICA_GROUPS,
                    ins=[vsend_dr[:]], outs=[vrecv_dr[:]],
                )
                nc.gpsimd.collective_compute(
                    "AllGather", OP.bypass, replica_groups=REPLICA_GROUPS,
                    ins=[vsend2_dr[:]], outs=[vrecv2_dr[:]],
                )
                nc.leave_named_scope(f"L{l:02d}_c_ag", None, False)
                nc.enter_named_scope(f"L{l:02d}_d_q", False)
                # ---- Q (overlaps the AllGather) ----
                for m in range(NB_D):
                    qp = psB.tile([P, T_OWN], F32, tag="psB", name=f"qps{l}_{m}")
                    mm_pairs(
                        qp[:],
                        lambda k, w: wq[:, k:k + w, m * P:(m + 1) * P],
                        lambda k, w: h1_fm[:, k:k + w, :],
                        NB_D,
                    )
                    for hh in range(2):
                        r0 = 64 * hh
                        nc.vector.tensor_scalar(
                            out=q_par[hh][r0:r0 + 64, m, :], in0=qp[r0:r0 + 64, :],
                            scalar1=bq_all[r0:r0 + 64, l, m:m + 1], scalar2=None,
                            op0=OP.add,
                        )

                # prefetch Wo while attention runs
                wo = wp.tile([P, NB_D, D], W8, tag="w8", bufs=4, name=f"wo{l}")
                nc.sync.dma_start(wo[:], wo_d[l].rearrange("p (k n) -> p k n", n=D))

                nc.leave_named_scope(f"L{l:02d}_d_q", None, False)
                nc.enter_named_scope(f"L{l:02d}_e_vrecv", False)
                # ---- receive V: augmented token-major (A8) + feature-major bf16 ----
                # own-relative key order: kb 0-3 = own half (local data),
                # kb 4-7 = peer half (dynamic shard pick from vrecv)
                # row stride padded to 784 (16-aligned) for DoubleRow LDWEIGHTS
                v_aug_full = ap.tile([P, NB_K, 784], A8, tag="v_aug", bufs=1,
                                     name=f"vaug{l}")
                v_aug = v_aug_full[:, :, 0:H * 65]
                va4 = v_aug.rearrange("p k (h c) -> p k h c", c=65)
                for kb in range(NB_K):
                    nc.vector.memset(va4[:, kb, :, 64:65], 1.0)
                for tb in range(NB_T):
                    nc.vector.tensor_copy(
                        va4[:, tb, :, 0:64],
                        v_send[:, tb, :].rearrange("p (h c) -> p h c", c=64),
                    )
                peer_off = (1 - (nc.gpsimd.partition_id() % 2)) * NB_T
                vr4 = vrecv_dr.rearrange("k p (h c) -> k p h c", c=64)
                for tb in range(NB_T):
                    nc.gpsimd.dma_start(
                        va4[:, 4 + tb, :, 0:64], vr4[ds(peer_off + tb, 1)],
                    )
                # peer half of v_fm arrives pre-transposed from the AllGather
                peer_off2 = (1 - (nc.gpsimd.partition_id() % 2)) * NB_D
                for db in range(NB_D):
                    nc.gpsimd.dma_start(
                        v_fm[:, db, T_OWN:], vrecv2_dr[ds(peer_off2 + db, 1)],
                    )
                nc.leave_named_scope(f"L{l:02d}_e_vrecv", None, False)
                nc.enter_named_scope(f"L{l:02d}_f_attn", False)
                # ---- attention: flat software pipeline over (head-pair, kb-pair)
                # steps; ctx lags scores by one step; db 0-1 own-half steps overlap
                # the AllGather ----
                ctx_n = ap.tile([P, NB_D, T_OWN], A8, tag="ctx_n", bufs=1,
                                name=f"ctxn{l}")
                OWN_PAIRS = [(0, 1), (2, 3)]
                PEER_PAIRS = [(4, 5), (6, 7)]

                ctxps = {}
                ex_own = {}
                pending = None  # (db, pair, ex, is_first_peer, is_last)

                def score_mms(db, pair, spv):
                    for o in range(2):
                        kb = pair[o]
                        for hh in range(2):
                            if SCORES_RT:
                                r0 = 64 * hh
                                nc.tensor.matmul(
                                    spv[hh][:, o * 512:(o + 1) * 512],
                                    v_fm[r0:r0 + 64, db, kb * P:(kb + 1) * P],
                                    q_par[hh][r0:r0 + 64, db, :],
                                    start=True, stop=True,
                                )
                            else:
                                nc.tensor.matmul(
                                    spv[hh][:, o * 512:(o + 1) * 512],
                                    v_fm[:, db, kb * P:(kb + 1) * P],
                                    q_par[hh][:, db, :],
                                    start=True, stop=True,
                                )

                def exp_tiles(l, db, pair, spv, tag, bufs):
                    ex = [
                        ap.tile([P, 2, 512], A8, tag=tag, bufs=bufs,
                                name=f"ex{l}_{db}_{pair[0]}_{hh}")
                        for hh in range(2)
                    ]
                    for hh in range(2):
                        nc.scalar.activation(
                            ex[hh].rearrange("p a b -> p (a b)"), spv[hh][:],
                            AF.Exp, scale=SCALE / (WS * WS),
                        )
                    return ex

                def ctx_mm(db, pair, ex, start, stop):
                    hpair = (2 * db, 2 * db + 1)
                    for hh in range(2):
                        h = hpair[hh]
                        if FP8_ATTN:
                            nc.tensor.matmul(
                                ctxps[db][hh][:],
                                v_aug[:, pair[0]:pair[0] + 2, 65 * h:65 * h + 65],
                                ex[hh][:],
                                start=start, stop=stop,
                                perf_mode=DR,
                            )
                        else:
                            for o in range(2):
                                nc.tensor.matmul(
                                    ctxps[db][hh][:],
                                    v_aug[:, pair[o], 65 * h:65 * h + 65],
                                    ex[hh][:, o, :],
                                    start=(start and o == 0), stop=(stop and o == 1),
                                )

                def flush_ctx():
                    nonlocal pending
                    if pending is None:
                        return
                    db, pair, ex, first, last = pending
                    if first:
                        ctxps[db] = [
                            psB.tile([65, T_OWN], F32, tag="psB",
                                     name=f"ctxp{l}_{2 * db + hh}")
                            for hh in range(2)
                        ]
                        for j, opair in enumerate(OWN_PAIRS):
                            ctx_mm(db, opair, ex_own.pop((db, opair)),
                                   start=(j == 0), stop=False)
                    ctx_mm(db, pair, ex, start=False, stop=last)
                    if last:
                        # Z rows -> bf16 sbuf, broadcast both heads into one
                        # [128,512] psum via two accumulating K=1 outer products
                        # on the PE, one 128-lane reciprocal, then one DVE
                        # multiply per head.
                        zrow = [
                            ap.tile([1, T_OWN], BF16, tag="zrow", bufs=4,
                                    name=f"zr{l}_{db}_{hh}")
                            for hh in range(2)
                        ]
                        for hh in range(2):
                            nc.vector.tensor_copy(
                                zrow[hh][:], ctxps[db][hh][64:65, :]
                            )
                        zbcp = psA.tile([P, T_OWN], F32, tag="psA",
                                        name=f"zbp{l}_{db}")
                        nc.tensor.matmul(
                            zbcp[:], sel_a[:], zrow[0][:],
                            start=True, stop=False,
                        )
                        nc.tensor.matmul(
                            zbcp[:], sel_b[:], zrow[1][:],
                            start=False, stop=True,
                        )
                        zbci = ap.tile([P, T_OWN], F32, tag="zbc", bufs=2,
                                       name=f"zb{l}_{db}")
                        nc.vector.reciprocal_approx_fast(zbci[:], zbcp[:])
                        for hh in range(2):
                            r0 = 64 * hh
                            nc.vector.tensor_tensor(
                                ctx_n[r0:r0 + 64, db, :], ctxps[db][hh][0:64, :],
                                zbci[r0:r0 + 64, :], OP.mult,
                            )
                        del ctxps[db]
                    pending = None

                # phase 1: own-half scores+exp for every head pair, buffered.
                # Runs during the AllGather window (needs only v_fm own half
                # and q_par), keeping the PE warm while the collective runs.
                for db in range(NB_D):
                    for pair in OWN_PAIRS:
                        spv = [
                            psA.tile([P, 1024], F32, tag="psA",
                                     name=f"sc{l}_{db}_{pair[0]}_{hh}")
                            for hh in range(2)
                        ]
                        score_mms(db, pair, spv)
                        ex_own[(db, pair)] = exp_tiles(l, db, pair, spv,
                                                       "expOwn", 24)

                # phase 2: peer-half scores+exp pipelined with ctx accumulation
                # (own-half ctx MMs issue from the buffered exps).
                for db in range(NB_D):
                    for pi, pair in enumerate(PEER_PAIRS):
                        spv = [
                            psA.tile([P, 1024], F32, tag="psA",
                                     name=f"sc{l}_{db}_{pair[0]}_{hh}")
                            for hh in range(2)
                        ]
                        score_mms(db, pair, spv)
                        flush_ctx()
                        ex = exp_tiles(l, db, pair, spv, "expT", 5)
                        pending = (db, pair, ex, pi == 0,
                                   pi == len(PEER_PAIRS) - 1)
                flush_ctx()
                nc.leave_named_scope(f"L{l:02d}_f_attn", None, False)
                nc.enter_named_scope(f"L{l:02d}_g_wo", False)
                # ---- Wo + residual; LN2 stats per block as it completes ----
                sqrt_prefetch(f"sqp2_{l}")
                skip = sp.tile([P, NB_T, D], F32R, tag="stream", name=f"skip{l}")
                mv4b = ap.tile([P, NB_T, 2], F32, tag="mv4", bufs=2, name=f"mv2_{l}")
                for lb in range(NB_T):
                    wps = psA.tile([P, D], F32, tag="psA", name=f"wops{l}_{lb}")
                    for n0, n1 in _regions():
                        mm_pairs(
                            wps[:, n0:n1],
                            lambda k, w: ctx_n[:, k:k + w, lb * P:(lb + 1) * P],
                            lambda k, w: wo[:, k:k + w, n0:n1],
                            NB_D,
                            extra_mm=None if zero_bias else (
                                lambda n0=n0, n1=n1: nc.tensor.matmul(
                                    wps[:, n0:n1], ones1[:], bo_row[:, n0:n1],
                                    start=False, stop=True,
                                )
                            ),
                        )
                    if FP8_ATTN:
                        nc.vector.scalar_tensor_tensor(
                            out=skip[:, lb, :], in0=wps[:], scalar=1.0 / (WS * WS),
                            in1=x_t[:, lb, :], op0=OP.mult, op1=OP.add,
                        )
                    else:
                        nc.vector.tensor_tensor(
                            skip[:, lb, :], x_t[:, lb, :], wps[:], OP.add
                        )
                    ln_stats(skip[:, lb, :], mv4b[:, lb, :])
                nc.leave_named_scope(f"L{l:02d}_g_wo", None, False)
                nc.enter_named_scope(f"L{l:02d}_h_ln2", False)
                # ---- LN2 apply + transpose ----
                h_tm2 = ap.tile([P, NB_T, D], BF16, tag="h_tm", bufs=1,
                                name=f"h2tm{l}")
                rstd2 = ln_rstd(mv4b, NB_T, f"b{l}")
                for lb in range(NB_T):
                    ln_apply(h_tm2[:, lb, :], skip[:, lb, :], mv4b[:, lb, :],
                             rstd2[:, lb:lb + 1])
                h2_fm = transpose_tm_to_fm(h_tm2, BF16, "h2_fm", f"h2fm{l}")
                nc.leave_named_scope(f"L{l:02d}_h_ln2", None, False)
                nc.enter_named_scope(f"L{l:02d}_i_ff", False)
                # ---- FFN: FF1 (all 24 hidden blocks) then FF2 in PSUM (bf16) ----
                g_all = ap.tile([P, NB_FF, T_OWN], BF16, tag="g", bufs=1, name=f"g{l}")
                w2cs = []
                for ck in range(4):
                    w1c = wp.tile([P, NB_D, D], BF16, tag="wbf", bufs=5,
                                  name=f"w1c{l}_{ck}")
                    nc.sync.dma_start(w1c[:], w1_d[l, ck].rearrange("p (k n) -> p k n", n=D))
                    w2c = wp.tile([P, NB_D, D], BF16, tag="wbf", bufs=5,
                                  name=f"w2c{l}_{ck}")
                    nc.sync.dma_start(w2c[:], w2_d[l, ck].rearrange("p (k n) -> p k n", n=D))
                    w2cs.append(w2c)
                    for mm in range(NB_D):
                        fp = psB.tile([P, T_OWN], F32, tag="psB", name=f"f1ps{l}_{ck}_{mm}")
                        for kb in range(NB_D):
                            nc.tensor.matmul(
                                fp[:], w1c[:, kb, mm * P:(mm + 1) * P], h2_fm[:, kb, :],
                                start=(kb == 0), stop=(kb == NB_D - 1),
                            )
                        nc.scalar.activation(
                            g_all[:, 6 * ck + mm, :], fp[:], AF.Gelu,
                            bias=b1_all[:, l, 6 * ck + mm:6 * ck + mm + 1], scale=1.0,
                        )
                sqrt_prefetch(f"sqp1_{l}")
                for half in range(2):
                    f2s = []
                    for lb in (2 * half, 2 * half + 1):
                        f2 = psA.tile([P, D], F32, tag="psA", name=f"f2ps{l}_{lb}")
                        f2s.append(f2)
                        for n0, n1 in _regions():
                            for ck in range(4):
                                for mm in range(NB_D):
                                    last = zero_bias and ck == 3 and mm == NB_D - 1
                                    nc.tensor.matmul(
                                        f2[:, n0:n1],
                                        g_all[:, 6 * ck + mm, lb * P:(lb + 1) * P],
                                        w2cs[ck][:, mm, n0:n1],
                                        start=(ck == 0 and mm == 0), stop=last,
                                    )
                            if not zero_bias:
                                nc.tensor.matmul(
                                    f2[:, n0:n1], ones1[:], b2_row[:, n0:n1],
                                    start=False, stop=True,
                                )
                    for i, lb in enumerate((2 * half, 2 * half + 1)):
                        nc.vector.tensor_tensor(
                            skip[:, lb, :], skip[:, lb, :], f2s[i][:], OP.add,
                        )
                nc.leave_named_scope(f"L{l:02d}_i_ff", None, False)
                x_t = skip

            nc.sync.dma_start(out_d[:], x_t[:])
    nc.compile()
    return nc


def _preprocess(inputs, n_layers):
    """Fold LN affine into projections; lay out weights for tile DMA."""
    f32 = np.float32
    L = n_layers
    Wq = np.asarray(inputs["Wq"], f32)[:L]
    Wv = np.asarray(inputs["Wv"], f32)[:L]
    Wo = np.asarray(inputs["Wo"], f32)[:L]
    W1 = np.asarray(inputs["W1"], f32)[:L]
    W2 = np.asarray(inputs["W2"], f32)[:L]
    g1 = np.asarray(inputs["ln1_g"], f32)[:L]
    b1ln = np.asarray(inputs["ln1_b"], f32)[:L]
    g2 = np.asarray(inputs["ln2_g"], f32)[:L]
    b2ln = np.asarray(inputs["ln2_b"], f32)[:L]
    bq = np.asarray(inputs["bq"], f32)[:L]
    bv = np.asarray(inputs["bv"], f32)[:L]
    bo = np.asarray(inputs["bo"], f32)[:L]
    b1 = np.asarray(inputs["b1"], f32)[:L]
    b2 = np.asarray(inputs["b2"], f32)[:L]

    Wq_eff = g1[:, :, None] * Wq
    bq_eff = bq + np.einsum("ld,ldo->lo", b1ln, Wq)
    Wv_eff = g1[:, :, None] * Wv
    bv_eff = bv + np.einsum("ld,ldo->lo", b1ln, Wv)
    W1_eff = g2[:, :, None] * W1
    b1_eff = b1 + np.einsum("ld,ldo->lo", b2ln, W1)

    def fm_weight(W):  # [L, D, D] -> [L, 128, 6*768] with [p, k, n]
        return np.ascontiguousarray(
            W.reshape(L, NB_D, P, D).transpose(0, 2, 1, 3).reshape(L, P, NB_D * D)
        )

    bf = ml_dtypes.bfloat16
    f8 = ml_dtypes.float8_e4m3
    w8 = f8 if FP8_ATTN else bf
    wq_h = fm_weight(Wq_eff * WS).astype(w8)
    wv_h = fm_weight(Wv_eff * WS).astype(w8)
    wo_h = fm_weight(Wo * WS).astype(w8)
    w1_h = np.ascontiguousarray(
        W1_eff.reshape(L, NB_D, P, 4, D).transpose(0, 3, 2, 1, 4).reshape(L, 4, P, NB_D * D)
    ).astype(bf)
    w2_h = np.ascontiguousarray(
        W2.reshape(L, 4, NB_D, P, D).transpose(0, 1, 3, 2, 4).reshape(L, 4, P, NB_D * D)
    ).astype(bf)
    bq_h = np.ascontiguousarray((bq_eff * WS).reshape(L, NB_D, P).transpose(2, 0, 1))
    bvf_h = np.ascontiguousarray((bv_eff * WS).reshape(L, NB_D, P).transpose(2, 0, 1))
    b1_h = np.ascontiguousarray(b1_eff.reshape(L, NB_FF, P).transpose(2, 0, 1))

    sel2 = np.zeros((2, P), f32)
    sel2[0, 0:64] = 1.0
    sel2[1, 64:128] = 1.0
    sel2 = sel2.astype(bf)

    return {
        "wq": wq_h, "wv": wv_h, "wo": wo_h, "w1": w1_h, "w2": w2_h,
        "bq": bq_h, "bvf": bvf_h, "b1": b1_h,
        "bv_row": np.ascontiguousarray(bv_eff[None] * WS),
        "bo_row": np.ascontiguousarray(bo[None] * WS * WS).astype(bf),
        "b2_row": np.ascontiguousarray(b2[None]).astype(bf),
        "identbf": np.eye(P).astype(bf),
        "ones1": np.ones((1, P)).astype(bf),
        "sel2": sel2,
    }


def kernel(**inputs) -> np.ndarray:
    n_layers = N_LAYERS
    zero_bias = not any(
        np.any(np.asarray(inputs[k])) for k in ("bv", "bo", "b2", "ln1_b")
    )
    key = ("nc", n_layers, zero_bias)
    if key not in _cached:
        _cached[key] = build(n_layers, zero_bias)
    nc = _cached[key]

    shared = _preprocess(inputs, n_layers)
    x = np.asarray(inputs["x"], np.float32)  # [4, 1024, 768]
    B, T, _ = x.shape

    in_maps = []
    for c in range(8):
        b, half = c // 2, c % 2
        x_own = x[b, half * T_OWN:(half + 1) * T_OWN]          # [512, 768]
        x_tile = np.ascontiguousarray(
            x_own.reshape(NB_T, P, D).transpose(1, 0, 2)        # [128, 4, 768]
        )
        in_maps.append({**shared, "x": x_tile})

    trace = bool(int(os.environ.get("KERNEL_TRACE", "0")))
    if trace:
        _register_ntff_hook()
    res = run_bass_kernel_spmd(nc, in_maps, core_ids=list(range(8)), trace=trace)
    global _last_results
    _last_results = res

    out = np.empty((B, T, D), dtype=np.float32)
    for c in range(8):
        b, half = c // 2, c % 2
        o = res.results[c]["out"]                               # [128, 4, 768]
        out[b, half * T_OWN:(half + 1) * T_OWN] = (
            o.transpose(1, 0, 2).reshape(T_OWN, D)
        )
    return out
